# revision 21
# baseline (speedup 1.0000x reference)
"""GAT (2-layer, PyG-style) on 8 Trainium2 NeuronCores.

Strategy (v2 — host-staged gather, two collective-free device programs)
----------------------------------------------------------------------
- Nodes are sharded across the 8 cores by dst (N/8 rows each). Edges are
  sorted by dst and packed into 128-dst windows; each window's edges are
  padded to 128-edge tiles (slot layout identical on all cores; per-core
  counts only differ in the padding).
- The host stages the per-edge operand tiles (the "gather"):
    program P1 gets xgT tiles  — x[src_e] columns, [128 feat x 128 edge],
    program P2 gets g2 tiles   — tab2[src_e] rows, [128 edge x CLS],
  and the per-slot alpha/dst-position planes. Attention coefficients are
  computed on the host exactly as in the v1 kernel (layer-1 alpha is a
  pure function of the inputs; layer-2 alpha comes from per-node a_s2 /
  a_d2 scalars returned by P1).
- Program P1, per 128-edge tile:
    g   = xgT_tile^T @ W1            (PE, psum)
    g4  = copy psum->sbuf bf16       (ACT, batched over 4 tiles)
    gw  = g4 * alpha1                (DVE tensor_tensor, pair-expanded
                                      alpha plane to hit the 2x mode)
    oh  = (iota == dpos)             (DVE tensor_scalar, 4x mode)
    aggT += gw^T @ oh                (PE, psum accum per 128-dst window)
  and per window: h1T = Relu(aggT + b1) (ACT, bias per partition), then
  tab2T = [v_s2 | v_d2 | W2]^T @ h1T (PE) which is written out: rows
  0, 1 are a_s2 / a_d2, rows 2..CLS+1 are the layer-2 table.
- Program P2, per tile: alpha-one-hot via one fused tensor_scalar
  ((iota == dpos) * alpha2), then out[d,:] += ahot^T @ g2_tile (PE).
  Host adds b2 and reassembles the full output.

All engine work stays on device; the host does input marshalling
(edge-index bookkeeping, fancy-indexed tile staging) and the softmax
normalization of the attention logits, as in v1.

Self-contained: all shapes/structure are derived from the actual inputs.
"""

import numpy as np
import ml_dtypes

import bass_rust
import concourse.bass as bass
import concourse.bacc as bacc
import concourse.mybir as mybir
from concourse.bass_utils import run_bass_kernel_spmd
from concourse.tile import TileContext, ScopedClock

# ----------------------------------------------------------------------------
# Workaround: this walrus build rejects >1 sync wait on a CTRL op, but the
# stock TileContext tail drain carries one wait per live proc. Split them
# across nofuse NOPs (one wait each).
# ----------------------------------------------------------------------------


def _patched_drain_and_barrier(self, tick_clock, wait_clock):
    nc = self.nc
    probe = nc.sync.nop(nofuse=True, hint="tail_drain_waits")
    wait_clock.add_sem_waits(probe.ins, ScopedClock({None: tick_clock.global_clock}))
    si = probe.ins.sync_info
    waits = list(si.on_wait) if si is not None else []
    if len(waits) > 1:
        probe.ins.sync_info = bass_rust.SyncInfo(on_wait=waits[:1], on_update=[])
        for i in range(1, len(waits)):
            n = nc.sync.nop(nofuse=True, hint=f"tail_drain_waits_{i}")
            n.ins.sync_info = bass_rust.SyncInfo(on_wait=waits[i : i + 1], on_update=[])
    nc.sync.drain()
    nc.all_engine_barrier()
    assert self.sems is not None
    popped = nc._tile_sem_poison_stack.pop()
    assert popped is self._sem_poison
    nc.clear_and_free_semaphores(list(self.sems.allocated().values()))
    nc.all_engine_barrier()


TileContext._drain_and_barrier = _patched_drain_and_barrier

MAX_WAITS = 1  # this walrus build rejects instructions with more sync waits


def _split_sync_waits(nc, max_waits=MAX_WAITS):
    """Hoist excess per-instruction sync waits onto standalone nofuse NOPs
    placed immediately before the instruction (same engine)."""
    n_new = 0
    for bbname, bassbb in list(nc._state.bb_map.items()):
        bb = bassbb.bb
        insts = list(bb.instructions)
        out = []
        changed = False
        for inst in insts:
            si = inst.sync_info
            if si is not None and len(si.on_wait) > max_waits:
                waits = list(si.on_wait)
                extra = waits[:-max_waits]
                for j in range(0, len(extra), max_waits):
                    nop = mybir.InstNoOp(
                        name=f"{inst.name}-w{n_new}",
                        engine=inst.engine,
                        bass_nofuse=True,
                        sync_info=bass_rust.SyncInfo(
                            on_wait=extra[j : j + max_waits], on_update=[]
                        ),
                    )
                    n_new += 1
                    nc.register_instruction(nop, overwrite=True)
                    out.append(nop)
                inst.sync_info = bass_rust.SyncInfo(
                    on_wait=waits[-max_waits:], on_update=list(si.on_update)
                )
                changed = True
            out.append(inst)
        if changed:
            bb.instructions = out
    return n_new

# ----------------------------------------------------------------------------

P = 128
N_CORES = 8
NEG_SLOPE = 0.2
BATCH = 8  # tiles per psum batch in P1 (psum tile spans 2 banks)
SLAB = 2  # chunks per input DMA slab in P1

F32 = mybir.dt.float32
BF16 = mybir.dt.bfloat16
BF = ml_dtypes.bfloat16

_CACHE = {}


def _leaky(z):
    return np.where(z > 0, z, NEG_SLOPE * z)


def _seg_softmax(z, dst, n):
    """Exact segment softmax over sorted dst (every dst has >=1 edge)."""
    starts = np.searchsorted(dst, np.arange(n))
    m = np.maximum.reduceat(z, starts, axis=0)
    w = np.exp(z - m[dst])
    den = np.add.reduceat(w, starts, axis=0)
    return w / den[dst]


class _Meta:
    pass


def _preprocess(N, edge_index):
    """Sort edges by dst, shard by dst range, build the static window/tile
    slot structure shared by both device programs (identical on all cores;
    per-core data differs only in padding)."""
    mt = _Meta()
    assert N % N_CORES == 0
    NLOC = N // N_CORES
    CH = (NLOC + P - 1) // P
    mt.N, mt.NLOC, mt.CH = N, NLOC, CH
    mt.SH_PAD = CH * P

    src = np.concatenate([edge_index[0], np.arange(N, dtype=np.int64)])
    dst = np.concatenate([edge_index[1], np.arange(N, dtype=np.int64)])
    order = np.argsort(dst, kind="stable")
    mt.src_s, mt.dst_s = src[order], dst[order]
    E = src.shape[0]
    mt.E = E

    # per-(core, window) edge ranges
    cnt = np.zeros((N_CORES, CH), dtype=np.int64)
    rng = np.zeros((N_CORES, CH, 2), dtype=np.int64)
    for c in range(N_CORES):
        for k in range(CH):
            d0 = c * NLOC + k * P
            d1 = min(c * NLOC + min((k + 1) * P, NLOC), N)
            s = np.searchsorted(mt.dst_s, d0)
            e = np.searchsorted(mt.dst_s, d1)
            rng[c, k] = (s, e)
            cnt[c, k] = e - s

    tiles_k = ((cnt.max(axis=0) + P - 1) // P).astype(np.int64)  # per window
    mt.tiles_k = tiles_k
    mt.T = int(tiles_k.sum())
    mt.S = mt.T * P
    mt.tile_off = np.concatenate([[0], np.cumsum(tiles_k)])[:-1]  # tile idx of win k

    # per-core slot arrays
    slot_src = np.zeros((N_CORES, mt.S), dtype=np.int64)
    slot_eid = np.full((N_CORES, mt.S), -1, dtype=np.int64)
    slot_dpos = np.full((N_CORES, mt.S), -1.0, dtype=np.float64)
    for c in range(N_CORES):
        for k in range(CH):
            s, e = rng[c, k]
            ne = e - s
            s0 = int(mt.tile_off[k]) * P
            slot_src[c, s0 : s0 + ne] = mt.src_s[s:e]
            slot_eid[c, s0 : s0 + ne] = np.arange(s, e)
            slot_dpos[c, s0 : s0 + ne] = mt.dst_s[s:e] - (c * NLOC + k * P)
    mt.slot_src = slot_src
    mt.slot_eid = slot_eid

    # dpos plane [C, 128, T] f32: slot j -> [j%128, j//128]
    mt.dpos_plane = np.ascontiguousarray(
        slot_dpos.reshape(N_CORES, mt.T, P).transpose(0, 2, 1)
    ).astype(np.float32)
    return mt


def _alpha_plane_pairs(mt, alpha):
    """alpha [E, H] (dst-sorted edge order) -> [C, 128, T, H, 2] bf16 slot
    planes with each value duplicated in pairs (so the DVE 2x mode applies)."""
    H = alpha.shape[1]
    eid = mt.slot_eid
    valid = eid >= 0
    vals = np.zeros((N_CORES, mt.S, H), dtype=np.float32)
    vals[valid] = alpha[eid[valid]].astype(np.float32)
    out = vals.reshape(N_CORES, mt.T, P, H).transpose(0, 2, 1, 3)  # [C,128,T,H]
    out = np.repeat(out[..., None], 2, axis=-1)  # [C,128,T,H,2]
    return np.ascontiguousarray(out).astype(BF)


def _onehot_batch(nc, oh_sb, iota_sb, dexp_sb, t0, nt):
    """oh[:, j, d] = (iota[d] == dpos[:, t0+j]) for j in [0, nt), one DVE op.
    All operands viewed with a trailing [1,2] pair dim so the 2x mode kicks
    in (dexp is the pair-duplicated dpos plane)."""
    d_ap = dexp_sb[:, t0 : t0 + nt, :]
    d_bc = bass.AP(
        d_ap.tensor,
        d_ap.offset,
        [list(d_ap.ap[0]), [2, nt], [0, P // 2], [1, 2]],
    )
    oh_ap = oh_sb[:, :nt, :]
    oh_v = bass.AP(
        oh_ap.tensor,
        oh_ap.offset,
        [list(oh_ap.ap[0]), [P, nt], [2, P // 2], [1, 2]],
    )
    io_ap = iota_sb[:]
    io_bc = bass.AP(
        io_ap.tensor,
        io_ap.offset,
        [list(io_ap.ap[0]), [0, nt], [2, P // 2], [1, 2]],
    )
    nc.vector.tensor_tensor(
        out=oh_v, in0=io_bc, in1=d_bc, op=mybir.AluOpType.is_equal
    )


def _build_p1(mt, F_IN, HEADS, HID, CLS):
    """Layer-1 program: xgT tiles -> h1 windows -> tab2T/asd output."""
    F_HID = HEADS * HID
    CH, T, S = mt.CH, mt.T, mt.S
    W2E = ((CLS + 2 + 15) // 16) * 16  # padded [v_s2 | v_d2 | W2] columns
    assert F_IN == P and F_HID == P

    nc = bacc.Bacc("TRN2", target_bir_lowering=False, debug=False, num_devices=N_CORES)
    xg_in = nc.declare_dram_parameter("xg", [P, S], BF16, isOutput=False)
    aexp_in = nc.declare_dram_parameter("aexp", [P, T * HEADS * 2], BF16, isOutput=False)
    dexp_in = nc.declare_dram_parameter("dexp", [P, T * 2], BF16, isOutput=False)
    iota_in = nc.declare_dram_parameter("iota", [P, P], BF16, isOutput=False)
    w1_in = nc.declare_dram_parameter("w1", [P, F_HID], BF16, isOutput=False)
    w2e_in = nc.declare_dram_parameter("w2e", [P, W2E], BF16, isOutput=False)
    b1_in = nc.declare_dram_parameter("b1col", [P, 1], F32, isOutput=False)
    houtT = nc.declare_dram_parameter("houtT", [W2E, CH * P], BF16, isOutput=True)
    asdT = nc.declare_dram_parameter("asdT", [2, CH * P], F32, isOutput=True)

    # resident-xg slab boundaries: ~8 big DMAs aligned to chunk starts
    slab_bounds = [0]
    tgt = (T + 7) // 8
    for k in range(CH):
        t_end = int(mt.tile_off[k]) + int(mt.tiles_k[k])
        if (t_end - slab_bounds[-1] >= tgt or k == CH - 1) and t_end > slab_bounds[-1]:
            slab_bounds.append(t_end)
    chunk_slab = {}
    for k in range(CH):
        t0 = int(mt.tile_off[k])
        chunk_slab[k] = sum(1 for b in slab_bounds[1:-1] if b <= t0)

    with TileContext(nc) as tc:
        with (
            tc.tile_pool(name="res", bufs=1) as res,
            tc.tile_pool(name="g4p", bufs=3) as g4p,
            tc.tile_pool(name="gwp", bufs=3) as gwp,
            tc.tile_pool(name="ohp", bufs=3) as ohp,
            tc.tile_pool(name="epi", bufs=3) as epi,
            tc.tile_pool(name="psg", bufs=2, space="PSUM") as psg,
            tc.tile_pool(name="psa", bufs=2, space="PSUM") as psa,
            tc.tile_pool(name="pst", bufs=2, space="PSUM") as pst,
        ):
            # residents on the ACT HWDGE ring; data slabs on the SP ring, so
            # the first slab isn't queued behind the big alpha plane.
            w1_sb = res.tile([P, F_HID], BF16)
            nc.scalar.dma_start(out=w1_sb[:], in_=w1_in[:])
            iota_sb = res.tile([P, P], BF16)
            nc.scalar.dma_start(out=iota_sb[:], in_=iota_in[:])
            b1_sb = res.tile([P, 1], F32)
            nc.scalar.dma_start(out=b1_sb[:], in_=b1_in[:])
            dexp_sb = res.tile([P, T, 2], BF16)
            nc.scalar.dma_start(
                out=dexp_sb[:],
                in_=dexp_in[:].rearrange("p (t two) -> p t two", two=2),
            )
            aexp_sb = res.tile([P, T, HEADS, 2], BF16)
            nc.scalar.dma_start(
                out=aexp_sb[:],
                in_=aexp_in[:].rearrange("p (t h two) -> p t h two", h=HEADS, two=2),
            )
            w2e_sb = res.tile([P, W2E], BF16)
            nc.scalar.dma_start(out=w2e_sb[:], in_=w2e_in[:])

            xg_slabs = []
            for si in range(len(slab_bounds) - 1):
                a, b = slab_bounds[si], slab_bounds[si + 1]
                sl = res.tile([P, b - a, P], BF16)
                nc.sync.dma_start(
                    out=sl[:],
                    in_=xg_in[:, a * P : b * P].rearrange("p (t e) -> p t e", e=P),
                )
                xg_slabs.append(sl)

            for k in range(CH):
                nt = int(mt.tiles_k[k])
                t0 = int(mt.tile_off[k])
                si = chunk_slab[k]
                xg_sb = xg_slabs[si][
                    :, t0 - slab_bounds[si] : t0 - slab_bounds[si] + nt, :
                ]
                agg_ps = psa.tile([P, P], F32, tag="agg")
                g_sb = g4p.tile([P, nt, P], BF16, tag="g")
                n_batches = (nt + BATCH - 1) // BATCH
                for b in range(n_batches):
                    j0 = b * BATCH
                    nb = min(BATCH, nt - j0)
                    g_ps = psg.tile([P, BATCH * P], F32, tag="gps")
                    for j in range(nb):
                        nc.tensor.matmul(
                            g_ps[:, j * P : (j + 1) * P],
                            lhsT=xg_sb[:, j0 + j, :],
                            rhs=w1_sb[:],
                            start=True,
                            stop=True,
                            skip_group_check=True,
                        )
                    nc.scalar.activation(
                        g_sb[:, j0 : j0 + nb, :],
                        g_ps[:, : nb * P].rearrange("p (t e) -> p t e", e=P),
                        mybir.ActivationFunctionType.Copy,
                    )
                # gw = g * alpha over the whole chunk (one DVE op, 2x mode)
                gw = gwp.tile([P, nt, P], BF16, tag="gw")
                a_ap = aexp_sb[:, t0 : t0 + nt, :, :]
                a_bc = bass.AP(
                    a_ap.tensor,
                    a_ap.offset,
                    [list(a_ap.ap[0]), [2, nt * HEADS], [0, HID // 2], [1, 2]],
                )
                gview = lambda tile: bass.AP(
                    tile.tensor,
                    tile.offset,
                    [list(tile.ap[0]), [HID, nt * HEADS], [2, HID // 2], [1, 2]],
                )
                nc.vector.tensor_tensor(
                    out=gview(gw[:, :, :]),
                    in0=gview(g_sb[:, :, :]),
                    in1=a_bc,
                    op=mybir.AluOpType.mult,
                )
                oh = ohp.tile([P, nt, P], BF16, tag="oh")
                _onehot_batch(nc, oh, iota_sb, dexp_sb, t0, nt)
                for j in range(nt):
                    nc.tensor.matmul(
                        agg_ps[:],
                        lhsT=gw[:, j, :],
                        rhs=oh[:, j, :],
                        start=(j == 0),
                        stop=(j == nt - 1),
                        skip_group_check=True,
                    )
                # epilogue: h1T = relu(aggT + b1), tab2T = W2e^T @ h1T
                h1T = epi.tile([P, P], BF16, tag="h1T")
                nc.scalar.activation(
                    h1T[:],
                    agg_ps[:],
                    mybir.ActivationFunctionType.Relu,
                    bias=b1_sb[:, 0:1],
                )
                t2_ps = pst.tile([W2E, P], F32, tag="t2")
                nc.tensor.matmul(
                    t2_ps[:],
                    lhsT=w2e_sb[:],
                    rhs=h1T[:],
                    start=True,
                    stop=True,
                    skip_group_check=True,
                )
                t2_sb = epi.tile([W2E, P], BF16, tag="t2sb")
                nc.scalar.activation(
                    t2_sb[:], t2_ps[:], mybir.ActivationFunctionType.Copy
                )
                asd_sb = epi.tile([2, P], F32, tag="asd")
                nc.vector.tensor_copy(out=asd_sb[:], in_=t2_ps[0:2, :])
                nc.sync.dma_start(out=houtT[:, k * P : (k + 1) * P], in_=t2_sb[:])
                nc.sync.dma_start(out=asdT[:, k * P : (k + 1) * P], in_=asd_sb[:])
    nc.compile()
    _split_sync_waits(nc)
    return nc


def _build_p2(mt, CLS):
    """Layer-2 program: g2 tiles * alpha2 aggregated per window (transposed
    matmul so the PE streams 128 columns and leaves the cold p-state)."""
    CH, T, S = mt.CH, mt.T, mt.S
    CLSP = ((CLS + 15) // 16) * 16

    nc = bacc.Bacc("TRN2", target_bir_lowering=False, debug=False, num_devices=N_CORES)
    g2_in = nc.declare_dram_parameter("g2", [P, T * CLSP], BF16, isOutput=False)
    aexp_in = nc.declare_dram_parameter("aexp", [P, T * 2], BF16, isOutput=False)
    dexp_in = nc.declare_dram_parameter("dexp", [P, T * 2], BF16, isOutput=False)
    iota_in = nc.declare_dram_parameter("iota", [P, P], BF16, isOutput=False)
    houtT = nc.declare_dram_parameter("houtT", [CLSP, CH * P], F32, isOutput=True)

    # resident-g2 slab boundaries: ~4 big DMAs aligned to chunk starts
    slab_bounds = [0]
    tgt = (T + 7) // 8
    for k in range(CH):
        t_end = int(mt.tile_off[k]) + int(mt.tiles_k[k])
        if (t_end - slab_bounds[-1] >= tgt or k == CH - 1) and t_end > slab_bounds[-1]:
            slab_bounds.append(t_end)
    chunk_slab = {}
    for k in range(CH):
        t0 = int(mt.tile_off[k])
        chunk_slab[k] = sum(1 for b in slab_bounds[1:-1] if b <= t0)

    with TileContext(nc) as tc:
        with (
            tc.tile_pool(name="res", bufs=1) as res,
            tc.tile_pool(name="gwp", bufs=3) as gwp,
            tc.tile_pool(name="ohp", bufs=3) as ohp,
            tc.tile_pool(name="epi", bufs=3) as epi,
            tc.tile_pool(name="pso", bufs=2, space="PSUM") as pso,
        ):
            iota_sb = res.tile([P, P], BF16)
            nc.scalar.dma_start(out=iota_sb[:], in_=iota_in[:])
            dexp_sb = res.tile([P, T, 2], BF16)
            nc.scalar.dma_start(
                out=dexp_sb[:],
                in_=dexp_in[:].rearrange("p (t two) -> p t two", two=2),
            )
            aexp_sb = res.tile([P, T, 2], BF16)
            nc.scalar.dma_start(
                out=aexp_sb[:],
                in_=aexp_in[:].rearrange("p (t two) -> p t two", two=2),
            )
            g2_slabs = []
            for si in range(len(slab_bounds) - 1):
                a, b = slab_bounds[si], slab_bounds[si + 1]
                sl = res.tile([P, b - a, CLSP], BF16)
                nc.sync.dma_start(
                    out=sl[:],
                    in_=g2_in[:, a * CLSP : b * CLSP].rearrange(
                        "p (t e) -> p t e", e=CLSP
                    ),
                )
                g2_slabs.append(sl)

            for k in range(CH):
                nt = int(mt.tiles_k[k])
                t0 = int(mt.tile_off[k])
                si = chunk_slab[k]
                g2_sb = g2_slabs[si][:, t0 - slab_bounds[si] : t0 - slab_bounds[si] + nt, :]
                # g2w = g2 * alpha2 (one DVE op per chunk, 2x mode)
                g2w = gwp.tile([P, nt, CLSP], BF16, tag="g2w")
                a_ap = aexp_sb[:, t0 : t0 + nt, :]
                a_bc = bass.AP(
                    a_ap.tensor,
                    a_ap.offset,
                    [list(a_ap.ap[0]), [2, nt], [0, CLSP // 2], [1, 2]],
                )
                gview = lambda tile: bass.AP(
                    tile.tensor,
                    tile.offset,
                    [list(tile.ap[0]), [CLSP, nt], [2, CLSP // 2], [1, 2]],
                )
                nc.vector.tensor_tensor(
                    out=gview(g2w[:, :, :]),
                    in0=gview(g2_sb[:, :, :]),
                    in1=a_bc,
                    op=mybir.AluOpType.mult,
                )
                oh = ohp.tile([P, nt, P], BF16, tag="oh")
                _onehot_batch(nc, oh, iota_sb, dexp_sb, t0, nt)
                o_ps = pso.tile([CLSP, P], F32, tag="o")
                for j in range(nt):
                    nc.tensor.matmul(
                        o_ps[:],
                        lhsT=g2w[:, j, :],
                        rhs=oh[:, j, :],
                        start=(j == 0),
                        stop=(j == nt - 1),
                        skip_group_check=True,
                    )
                o_sb = epi.tile([CLSP, P], F32, tag="osb")
                nc.scalar.activation(
                    o_sb[:], o_ps[:], mybir.ActivationFunctionType.Copy
                )
                nc.sync.dma_start(out=houtT[:, k * P : (k + 1) * P], in_=o_sb[:])
    nc.compile()
    _split_sync_waits(nc)
    return nc


def kernel(
    x,
    edge_index,
    W1,
    att_src1,
    att_dst1,
    b1,
    W2,
    att_src2,
    att_dst2,
    b2,
    _trace=False,
    _tmpdirs=None,
):
    x = np.asarray(x, dtype=np.float32)
    edge_index = np.asarray(edge_index).astype(np.int64)
    W1 = np.asarray(W1, dtype=np.float32)
    att_src1 = np.asarray(att_src1, dtype=np.float32)
    att_dst1 = np.asarray(att_dst1, dtype=np.float32)
    b1 = np.asarray(b1, dtype=np.float32)
    W2 = np.asarray(W2, dtype=np.float32)
    att_src2 = np.asarray(att_src2, dtype=np.float32)
    att_dst2 = np.asarray(att_dst2, dtype=np.float32)
    b2 = np.asarray(b2, dtype=np.float32)

    N, F_IN = x.shape
    HEADS, HID = att_src1.shape
    CLS = W2.shape[1]
    W2E = ((CLS + 2 + 15) // 16) * 16
    CLSP = ((CLS + 15) // 16) * 16

    key = (N, edge_index.shape[1], F_IN, HEADS, HID, CLS, hash(edge_index.tobytes()))
    if key in _CACHE:
        mt, ncA, ncB = _CACHE[key]
    else:
        mt = _preprocess(N, edge_index)
        ncA = _build_p1(mt, F_IN, HEADS, HID, CLS)
        ncB = _build_p2(mt, CLS)
        _CACHE[key] = (mt, ncA, ncB)

    NLOC, CH, T, S = mt.NLOC, mt.CH, mt.T, mt.S

    # ---- host: layer-1 alpha (a_s/a_d are linear in x) ----
    W1r = W1.reshape(F_IN, HEADS, HID)
    v_s = np.einsum("fhc,hc->fh", W1r, att_src1)
    v_d = np.einsum("fhc,hc->fh", W1r, att_dst1)
    a_s = x.astype(np.float64) @ v_s.astype(np.float64)
    a_d = x.astype(np.float64) @ v_d.astype(np.float64)
    z1 = _leaky(a_s[mt.src_s] + a_d[mt.dst_s])
    alpha1 = _seg_softmax(z1, mt.dst_s, N)

    aexp = _alpha_plane_pairs(mt, alpha1)  # [C,128,T,H,2] bf16
    iota = np.tile(np.arange(P, dtype=np.float32)[None, :], (P, 1)).astype(BF)
    dexp = np.ascontiguousarray(
        np.repeat(mt.dpos_plane[..., None], 2, axis=-1)
    ).astype(BF)  # [C,128,T,2]

    # xgT tiles: [C, 128 feat, S] bf16 = x.T columns at slot srcs
    xT16 = np.ascontiguousarray(x.astype(BF).T)  # [F_IN, N]
    w1b = W1.astype(BF)
    v_s2 = (W2 @ att_src2[0]).astype(np.float32)
    v_d2 = (W2 @ att_dst2[0]).astype(np.float32)
    w2e = np.zeros((HEADS * HID, W2E), np.float32)
    w2e[:, 0] = v_s2
    w2e[:, 1] = v_d2
    w2e[:, 2 : 2 + CLS] = W2
    w2eb = w2e.astype(BF)
    b1col = b1.reshape(P, 1).astype(np.float32)

    in_maps_a = []
    for c in range(N_CORES):
        xg = np.ascontiguousarray(xT16[:, mt.slot_src[c]])  # [128, S]
        in_maps_a.append(
            {
                "xg": xg,
                "aexp": np.ascontiguousarray(aexp[c].reshape(P, -1)),
                "dexp": np.ascontiguousarray(dexp[c].reshape(P, -1)),
                "iota": iota,
                "w1": w1b,
                "w2e": w2eb,
                "b1col": b1col,
            }
        )

    tds = _tmpdirs or [None, None]
    resA = run_bass_kernel_spmd(
        ncA, in_maps_a, list(range(N_CORES)), trace=_trace, tmpdir=tds[0]
    )

    # host: assemble tab2 + a_s2/a_d2, compute alpha2
    tab2 = np.zeros((N, CLSP), BF)
    asd = np.zeros((N, 2), np.float64)
    for c in range(N_CORES):
        hT = np.asarray(resA.results[c]["houtT"])  # [W2E, CH*P] bf16
        tab2[c * NLOC : (c + 1) * NLOC, :CLS] = hT[2 : 2 + CLS, :NLOC].T
        asd[c * NLOC : (c + 1) * NLOC] = np.asarray(
            resA.results[c]["asdT"], np.float64
        )[:, :NLOC].T

    z2 = _leaky(asd[mt.src_s, 0] + asd[mt.dst_s, 1])[:, None]
    alpha2 = _seg_softmax(z2, mt.dst_s, N)
    a2exp = _alpha_plane_pairs(mt, alpha2)  # [C,128,T,1,2]

    in_maps_b = []
    for c in range(N_CORES):
        g2 = tab2[mt.slot_src[c]]  # [S, CLSP] bf16
        g2 = np.ascontiguousarray(
            g2.reshape(T, P, CLSP).transpose(1, 0, 2).reshape(P, T * CLSP)
        )
        in_maps_b.append(
            {
                "g2": g2,
                "aexp": np.ascontiguousarray(a2exp[c].reshape(P, -1)),
                "dexp": np.ascontiguousarray(dexp[c].reshape(P, -1)),
                "iota": iota,
            }
        )

    resB = run_bass_kernel_spmd(
        ncB, in_maps_b, list(range(N_CORES)), trace=_trace, tmpdir=tds[1]
    )

    out = np.zeros((N, CLS), np.float32)
    for c in range(N_CORES):
        out[c * NLOC : (c + 1) * NLOC] = np.asarray(
            resB.results[c]["houtT"], np.float32
        )[:CLS, :NLOC].T
    out += b2[None, :]

    kernel._last = (resA, resB)
    return out


# revision 25
# speedup vs baseline: 1.2290x; 1.2290x over previous
"""GAT (2-layer, PyG-style) on 8 Trainium2 NeuronCores.

Strategy (v2 — host-staged gather, two collective-free device programs)
----------------------------------------------------------------------
- Nodes are sharded across the 8 cores by dst (N/8 rows each). Edges are
  sorted by dst and packed into 128-dst windows; each window's edges are
  padded to 128-edge tiles (slot layout identical on all cores; per-core
  counts only differ in the padding).
- The host stages the per-edge operand tiles (the "gather"):
    program P1 gets xgT tiles  — x[src_e] columns, [128 feat x 128 edge],
    program P2 gets g2 tiles   — tab2[src_e] rows, [128 edge x CLS],
  and the per-slot alpha/dst-position planes. Attention coefficients are
  computed on the host exactly as in the v1 kernel (layer-1 alpha is a
  pure function of the inputs; layer-2 alpha comes from per-node a_s2 /
  a_d2 scalars returned by P1).
- Program P1, per 128-edge tile:
    g   = xgT_tile^T @ W1            (PE, psum)
    g4  = copy psum->sbuf bf16       (ACT, batched over 4 tiles)
    gw  = g4 * alpha1                (DVE tensor_tensor, pair-expanded
                                      alpha plane to hit the 2x mode)
    oh  = (iota == dpos)             (DVE tensor_scalar, 4x mode)
    aggT += gw^T @ oh                (PE, psum accum per 128-dst window)
  and per window: h1T = Relu(aggT + b1) (ACT, bias per partition), then
  tab2T = [v_s2 | v_d2 | W2]^T @ h1T (PE) which is written out: rows
  0, 1 are a_s2 / a_d2, rows 2..CLS+1 are the layer-2 table.
- Program P2, per tile: alpha-one-hot via one fused tensor_scalar
  ((iota == dpos) * alpha2), then out[d,:] += ahot^T @ g2_tile (PE).
  Host adds b2 and reassembles the full output.

All engine work stays on device; the host does input marshalling
(edge-index bookkeeping, fancy-indexed tile staging) and the softmax
normalization of the attention logits, as in v1.

Self-contained: all shapes/structure are derived from the actual inputs.
"""

import numpy as np
import ml_dtypes

import bass_rust
import concourse.bass as bass
import concourse.bacc as bacc
import concourse.mybir as mybir
from concourse.bass_utils import run_bass_kernel_spmd
from concourse.tile import TileContext, ScopedClock

# ----------------------------------------------------------------------------
# Workaround: this walrus build rejects >1 sync wait on a CTRL op, but the
# stock TileContext tail drain carries one wait per live proc. Split them
# across nofuse NOPs (one wait each).
# ----------------------------------------------------------------------------


def _patched_drain_and_barrier(self, tick_clock, wait_clock):
    nc = self.nc
    probe = nc.sync.nop(nofuse=True, hint="tail_drain_waits")
    wait_clock.add_sem_waits(probe.ins, ScopedClock({None: tick_clock.global_clock}))
    si = probe.ins.sync_info
    waits = list(si.on_wait) if si is not None else []
    if len(waits) > 1:
        probe.ins.sync_info = bass_rust.SyncInfo(on_wait=waits[:1], on_update=[])
        for i in range(1, len(waits)):
            n = nc.sync.nop(nofuse=True, hint=f"tail_drain_waits_{i}")
            n.ins.sync_info = bass_rust.SyncInfo(on_wait=waits[i : i + 1], on_update=[])
    nc.sync.drain()
    nc.all_engine_barrier()
    assert self.sems is not None
    popped = nc._tile_sem_poison_stack.pop()
    assert popped is self._sem_poison
    nc.clear_and_free_semaphores(list(self.sems.allocated().values()))
    nc.all_engine_barrier()


TileContext._drain_and_barrier = _patched_drain_and_barrier

MAX_WAITS = 1  # this walrus build rejects instructions with more sync waits


def _split_sync_waits(nc, max_waits=MAX_WAITS):
    """Hoist excess per-instruction sync waits onto standalone nofuse NOPs
    placed immediately before the instruction (same engine)."""
    n_new = 0
    for bbname, bassbb in list(nc._state.bb_map.items()):
        bb = bassbb.bb
        insts = list(bb.instructions)
        out = []
        changed = False
        for inst in insts:
            si = inst.sync_info
            if si is not None and len(si.on_wait) > max_waits:
                waits = list(si.on_wait)
                extra = waits[:-max_waits]
                for j in range(0, len(extra), max_waits):
                    nop = mybir.InstNoOp(
                        name=f"{inst.name}-w{n_new}",
                        engine=inst.engine,
                        bass_nofuse=True,
                        sync_info=bass_rust.SyncInfo(
                            on_wait=extra[j : j + max_waits], on_update=[]
                        ),
                    )
                    n_new += 1
                    nc.register_instruction(nop, overwrite=True)
                    out.append(nop)
                inst.sync_info = bass_rust.SyncInfo(
                    on_wait=waits[-max_waits:], on_update=list(si.on_update)
                )
                changed = True
            out.append(inst)
        if changed:
            bb.instructions = out
    return n_new

# ----------------------------------------------------------------------------

P = 128
N_CORES = 8
NEG_SLOPE = 0.2
BATCH = 8  # tiles per psum batch in P1 (psum tile spans 2 banks)
SLAB = 2  # chunks per input DMA slab in P1

F32 = mybir.dt.float32
BF16 = mybir.dt.bfloat16
BF = ml_dtypes.bfloat16

_CACHE = {}


def _leaky(z):
    return np.where(z > 0, z, NEG_SLOPE * z)


def _seg_softmax(z, dst, n):
    """Exact segment softmax over sorted dst (every dst has >=1 edge)."""
    starts = np.searchsorted(dst, np.arange(n))
    m = np.maximum.reduceat(z, starts, axis=0)
    w = np.exp(z - m[dst])
    den = np.add.reduceat(w, starts, axis=0)
    return w / den[dst]


class _Meta:
    pass


def _preprocess(N, edge_index):
    """Sort edges by dst, shard by dst range, build the static window/tile
    slot structure shared by both device programs (identical on all cores;
    per-core data differs only in padding)."""
    mt = _Meta()
    assert N % N_CORES == 0
    NLOC = N // N_CORES
    CH = (NLOC + P - 1) // P
    mt.N, mt.NLOC, mt.CH = N, NLOC, CH
    mt.SH_PAD = CH * P

    src = np.concatenate([edge_index[0], np.arange(N, dtype=np.int64)])
    dst = np.concatenate([edge_index[1], np.arange(N, dtype=np.int64)])
    order = np.argsort(dst, kind="stable")
    mt.src_s, mt.dst_s = src[order], dst[order]
    E = src.shape[0]
    mt.E = E

    # per-(core, window) edge ranges
    cnt = np.zeros((N_CORES, CH), dtype=np.int64)
    rng = np.zeros((N_CORES, CH, 2), dtype=np.int64)
    for c in range(N_CORES):
        for k in range(CH):
            d0 = c * NLOC + k * P
            d1 = min(c * NLOC + min((k + 1) * P, NLOC), N)
            s = np.searchsorted(mt.dst_s, d0)
            e = np.searchsorted(mt.dst_s, d1)
            rng[c, k] = (s, e)
            cnt[c, k] = e - s

    tiles_k = ((cnt.max(axis=0) + P - 1) // P).astype(np.int64)  # per window
    mt.tiles_k = tiles_k
    mt.T = int(tiles_k.sum())
    mt.S = mt.T * P
    mt.tile_off = np.concatenate([[0], np.cumsum(tiles_k)])[:-1]  # tile idx of win k

    # per-core slot arrays
    slot_src = np.zeros((N_CORES, mt.S), dtype=np.int64)
    slot_eid = np.full((N_CORES, mt.S), -1, dtype=np.int64)
    slot_dpos = np.full((N_CORES, mt.S), -1.0, dtype=np.float64)
    for c in range(N_CORES):
        for k in range(CH):
            s, e = rng[c, k]
            ne = e - s
            s0 = int(mt.tile_off[k]) * P
            slot_src[c, s0 : s0 + ne] = mt.src_s[s:e]
            slot_eid[c, s0 : s0 + ne] = np.arange(s, e)
            slot_dpos[c, s0 : s0 + ne] = mt.dst_s[s:e] - (c * NLOC + k * P)
    mt.slot_src = slot_src
    mt.slot_eid = slot_eid

    # dpos plane [C, 128, T] f32: slot j -> [j%128, j//128]
    mt.dpos_plane = np.ascontiguousarray(
        slot_dpos.reshape(N_CORES, mt.T, P).transpose(0, 2, 1)
    ).astype(np.float32)
    return mt


def _alpha_plane_pairs(mt, alpha):
    """alpha [E, H] (dst-sorted edge order) -> [C, 128, T, H, 2] bf16 slot
    planes with each value duplicated in pairs (so the DVE 2x mode applies)."""
    H = alpha.shape[1]
    eid = mt.slot_eid
    valid = eid >= 0
    vals = np.zeros((N_CORES, mt.S, H), dtype=np.float32)
    vals[valid] = alpha[eid[valid]].astype(np.float32)
    out = vals.reshape(N_CORES, mt.T, P, H).transpose(0, 2, 1, 3)  # [C,128,T,H]
    out = np.repeat(out[..., None], 2, axis=-1)  # [C,128,T,H,2]
    return np.ascontiguousarray(out).astype(BF)


def _onehot_batch(nc, oh_sb, iota_sb, dexp_sb, t0, nt):
    """oh[:, j, d] = (iota[d] == dpos[:, t0+j]) for j in [0, nt), one DVE op.
    All operands viewed with a trailing [1,2] pair dim so the 2x mode kicks
    in (dexp is the pair-duplicated dpos plane)."""
    d_ap = dexp_sb[:, t0 : t0 + nt, :]
    d_bc = bass.AP(
        d_ap.tensor,
        d_ap.offset,
        [list(d_ap.ap[0]), [2, nt], [0, P // 2], [1, 2]],
    )
    oh_ap = oh_sb[:, :nt, :]
    oh_v = bass.AP(
        oh_ap.tensor,
        oh_ap.offset,
        [list(oh_ap.ap[0]), [P, nt], [2, P // 2], [1, 2]],
    )
    io_ap = iota_sb[:]
    io_bc = bass.AP(
        io_ap.tensor,
        io_ap.offset,
        [list(io_ap.ap[0]), [0, nt], [2, P // 2], [1, 2]],
    )
    nc.vector.tensor_tensor(
        out=oh_v, in0=io_bc, in1=d_bc, op=mybir.AluOpType.is_equal
    )


def _build_p1(mt, F_IN, HEADS, HID, CLS):
    """Layer-1 program: xgT tiles -> h1 windows -> tab2T/asd output."""
    F_HID = HEADS * HID
    CH, T, S = mt.CH, mt.T, mt.S
    W2E = ((CLS + 2 + 15) // 16) * 16  # padded [v_s2 | v_d2 | W2] columns
    assert F_IN == P and F_HID == P

    nc = bacc.Bacc("TRN2", target_bir_lowering=False, debug=False, num_devices=N_CORES)
    xg_in = nc.declare_dram_parameter("xg", [P, S], BF16, isOutput=False)
    aexp_in = nc.declare_dram_parameter("aexp", [P, T * HEADS * 2], BF16, isOutput=False)
    dexp_in = nc.declare_dram_parameter("dexp", [P, T * 2], BF16, isOutput=False)
    iota_in = nc.declare_dram_parameter("iota", [P, P], BF16, isOutput=False)
    w1_in = nc.declare_dram_parameter("w1", [P, F_HID], BF16, isOutput=False)
    w2e_in = nc.declare_dram_parameter("w2e", [P, W2E], BF16, isOutput=False)
    b1_in = nc.declare_dram_parameter("b1col", [P, 1], F32, isOutput=False)
    houtT = nc.declare_dram_parameter("houtT", [W2E, CH * P], BF16, isOutput=True)
    asdT = nc.declare_dram_parameter("asdT", [2, CH * P], F32, isOutput=True)

    # streamed-xg slab boundaries: ~16 DMAs, 4 rotating SBUF slots
    slab_bounds = [0]
    tgt = (T + 15) // 16
    for k in range(CH):
        t_end = int(mt.tile_off[k]) + int(mt.tiles_k[k])
        if (t_end - slab_bounds[-1] >= tgt or k == CH - 1) and t_end > slab_bounds[-1]:
            slab_bounds.append(t_end)
    chunk_slab = {}
    for k in range(CH):
        t0 = int(mt.tile_off[k])
        chunk_slab[k] = sum(1 for b in slab_bounds[1:-1] if b <= t0)

    with TileContext(nc) as tc:
        with (
            tc.tile_pool(name="res", bufs=1) as res,
            tc.tile_pool(name="g4p", bufs=3) as g4p,
            tc.tile_pool(name="gwp", bufs=3) as gwp,
            tc.tile_pool(name="ohp", bufs=3) as ohp,
            tc.tile_pool(name="epi", bufs=3) as epi,
            tc.tile_pool(name="psg", bufs=2, space="PSUM") as psg,
            tc.tile_pool(name="psa", bufs=2, space="PSUM") as psa,
            tc.tile_pool(name="pst", bufs=2, space="PSUM") as pst,
        ):
            # residents on the ACT HWDGE ring; data slabs on the SP ring, so
            # the first slab isn't queued behind the big alpha plane.
            w1_sb = res.tile([P, F_HID], BF16)
            nc.scalar.dma_start(out=w1_sb[:], in_=w1_in[:])
            iota_sb = res.tile([P, P], BF16)
            nc.scalar.dma_start(out=iota_sb[:], in_=iota_in[:])
            b1_sb = res.tile([P, 1], F32)
            nc.scalar.dma_start(out=b1_sb[:], in_=b1_in[:])
            dexp_sb = res.tile([P, T, 2], BF16)
            nc.scalar.dma_start(
                out=dexp_sb[:],
                in_=dexp_in[:].rearrange("p (t two) -> p t two", two=2),
            )
            aexp_sb = res.tile([P, T, HEADS, 2], BF16)
            nc.scalar.dma_start(
                out=aexp_sb[:],
                in_=aexp_in[:].rearrange("p (t h two) -> p t h two", h=HEADS, two=2),
            )
            w2e_sb = res.tile([P, W2E], BF16)
            nc.scalar.dma_start(out=w2e_sb[:], in_=w2e_in[:])

            max_slab = max(
                slab_bounds[i + 1] - slab_bounds[i]
                for i in range(len(slab_bounds) - 1)
            )
            xg_slabs = []
            for si in range(len(slab_bounds) - 1):
                a, b = slab_bounds[si], slab_bounds[si + 1]
                sl = res.tile(
                    [P, max_slab, P], BF16, name=f"xgsl{si}", tag=f"xgsl{si % 4}"
                )[:, : b - a, :]
                nc.sync.dma_start(
                    out=sl[:],
                    in_=xg_in[:, a * P : b * P].rearrange("p (t e) -> p t e", e=P),
                )
                xg_slabs.append(sl)

            for k in range(CH):
                nt = int(mt.tiles_k[k])
                t0 = int(mt.tile_off[k])
                si = chunk_slab[k]
                xg_sb = xg_slabs[si][
                    :, t0 - slab_bounds[si] : t0 - slab_bounds[si] + nt, :
                ]
                agg_ps = psa.tile([P, P], F32, tag="agg")
                g_sb = g4p.tile([P, nt, P], BF16, tag="g")
                n_batches = (nt + BATCH - 1) // BATCH
                for b in range(n_batches):
                    j0 = b * BATCH
                    nb = min(BATCH, nt - j0)
                    g_ps = psg.tile([P, BATCH * P], F32, tag="gps")
                    for j in range(nb):
                        nc.tensor.matmul(
                            g_ps[:, j * P : (j + 1) * P],
                            lhsT=xg_sb[:, j0 + j, :],
                            rhs=w1_sb[:],
                            start=True,
                            stop=True,
                            skip_group_check=True,
                        )
                    nc.scalar.activation(
                        g_sb[:, j0 : j0 + nb, :],
                        g_ps[:, : nb * P].rearrange("p (t e) -> p t e", e=P),
                        mybir.ActivationFunctionType.Copy,
                    )
                # gw = g * alpha over the whole chunk (one DVE op, 2x mode)
                gw = gwp.tile([P, nt, P], BF16, tag="gw")
                a_ap = aexp_sb[:, t0 : t0 + nt, :, :]
                a_bc = bass.AP(
                    a_ap.tensor,
                    a_ap.offset,
                    [list(a_ap.ap[0]), [2, nt * HEADS], [0, HID // 2], [1, 2]],
                )
                gview = lambda tile: bass.AP(
                    tile.tensor,
                    tile.offset,
                    [list(tile.ap[0]), [HID, nt * HEADS], [2, HID // 2], [1, 2]],
                )
                nc.vector.tensor_tensor(
                    out=gview(gw[:, :, :]),
                    in0=gview(g_sb[:, :, :]),
                    in1=a_bc,
                    op=mybir.AluOpType.mult,
                )
                oh = ohp.tile([P, nt, P], BF16, tag="oh")
                _onehot_batch(nc, oh, iota_sb, dexp_sb, t0, nt)
                for j in range(nt):
                    nc.tensor.matmul(
                        agg_ps[:],
                        lhsT=gw[:, j, :],
                        rhs=oh[:, j, :],
                        start=(j == 0),
                        stop=(j == nt - 1),
                        skip_group_check=True,
                    )
                # epilogue: h1T = relu(aggT + b1), tab2T = W2e^T @ h1T
                h1T = epi.tile([P, P], BF16, tag="h1T")
                nc.scalar.activation(
                    h1T[:],
                    agg_ps[:],
                    mybir.ActivationFunctionType.Relu,
                    bias=b1_sb[:, 0:1],
                )
                t2_ps = pst.tile([W2E, P], F32, tag="t2")
                nc.tensor.matmul(
                    t2_ps[:],
                    lhsT=w2e_sb[:],
                    rhs=h1T[:],
                    start=True,
                    stop=True,
                    skip_group_check=True,
                )
                t2_sb = epi.tile([W2E, P], BF16, tag="t2sb")
                nc.scalar.activation(
                    t2_sb[:], t2_ps[:], mybir.ActivationFunctionType.Copy
                )
                asd_sb = epi.tile([2, P], F32, tag="asd")
                nc.vector.tensor_copy(out=asd_sb[:], in_=t2_ps[0:2, :])
                nc.sync.dma_start(out=houtT[:, k * P : (k + 1) * P], in_=t2_sb[:])
                nc.sync.dma_start(out=asdT[:, k * P : (k + 1) * P], in_=asd_sb[:])
    nc.compile()
    _split_sync_waits(nc)
    return nc


def _build_p2(mt, CLS):
    """Layer-2 program: g2 tiles * alpha2 aggregated per window (transposed
    matmul so the PE streams 128 columns and leaves the cold p-state)."""
    CH, T, S = mt.CH, mt.T, mt.S
    CLSP = ((CLS + 15) // 16) * 16

    nc = bacc.Bacc("TRN2", target_bir_lowering=False, debug=False, num_devices=N_CORES)
    g2_in = nc.declare_dram_parameter("g2", [P, T * CLSP], BF16, isOutput=False)
    aexp_in = nc.declare_dram_parameter("aexp", [P, T * 2], BF16, isOutput=False)
    dexp_in = nc.declare_dram_parameter("dexp", [P, T * 2], BF16, isOutput=False)
    iota_in = nc.declare_dram_parameter("iota", [P, P], BF16, isOutput=False)
    houtT = nc.declare_dram_parameter("houtT", [CLSP, CH * P], F32, isOutput=True)

    # resident-g2 slab boundaries: ~4 big DMAs aligned to chunk starts
    slab_bounds = [0]
    tgt = (T + 7) // 8
    for k in range(CH):
        t_end = int(mt.tile_off[k]) + int(mt.tiles_k[k])
        if (t_end - slab_bounds[-1] >= tgt or k == CH - 1) and t_end > slab_bounds[-1]:
            slab_bounds.append(t_end)
    chunk_slab = {}
    for k in range(CH):
        t0 = int(mt.tile_off[k])
        chunk_slab[k] = sum(1 for b in slab_bounds[1:-1] if b <= t0)

    with TileContext(nc) as tc:
        with (
            tc.tile_pool(name="res", bufs=1) as res,
            tc.tile_pool(name="gwp", bufs=3) as gwp,
            tc.tile_pool(name="ohp", bufs=3) as ohp,
            tc.tile_pool(name="epi", bufs=3) as epi,
            tc.tile_pool(name="pso", bufs=2, space="PSUM") as pso,
        ):
            iota_sb = res.tile([P, P], BF16)
            nc.scalar.dma_start(out=iota_sb[:], in_=iota_in[:])
            dexp_sb = res.tile([P, T, 2], BF16)
            nc.scalar.dma_start(
                out=dexp_sb[:],
                in_=dexp_in[:].rearrange("p (t two) -> p t two", two=2),
            )
            aexp_sb = res.tile([P, T, 2], BF16)
            nc.scalar.dma_start(
                out=aexp_sb[:],
                in_=aexp_in[:].rearrange("p (t two) -> p t two", two=2),
            )
            g2_slabs = []
            for si in range(len(slab_bounds) - 1):
                a, b = slab_bounds[si], slab_bounds[si + 1]
                sl = res.tile([P, b - a, CLSP], BF16, name=f"g2sl{si}", tag=f"g2sl{si}")
                nc.sync.dma_start(
                    out=sl[:],
                    in_=g2_in[:, a * CLSP : b * CLSP].rearrange(
                        "p (t e) -> p t e", e=CLSP
                    ),
                )
                g2_slabs.append(sl)

            for k in range(CH):
                nt = int(mt.tiles_k[k])
                t0 = int(mt.tile_off[k])
                si = chunk_slab[k]
                g2_sb = g2_slabs[si][:, t0 - slab_bounds[si] : t0 - slab_bounds[si] + nt, :]
                # g2w = g2 * alpha2 (one DVE op per chunk, 2x mode)
                g2w = gwp.tile([P, nt, CLSP], BF16, tag="g2w")
                a_ap = aexp_sb[:, t0 : t0 + nt, :]
                a_bc = bass.AP(
                    a_ap.tensor,
                    a_ap.offset,
                    [list(a_ap.ap[0]), [2, nt], [0, CLSP // 2], [1, 2]],
                )
                gview = lambda tile: bass.AP(
                    tile.tensor,
                    tile.offset,
                    [list(tile.ap[0]), [CLSP, nt], [2, CLSP // 2], [1, 2]],
                )
                nc.vector.tensor_tensor(
                    out=gview(g2w[:, :, :]),
                    in0=gview(g2_sb[:, :, :]),
                    in1=a_bc,
                    op=mybir.AluOpType.mult,
                )
                oh = ohp.tile([P, nt, P], BF16, tag="oh")
                _onehot_batch(nc, oh, iota_sb, dexp_sb, t0, nt)
                o_ps = pso.tile([CLSP, P], F32, tag="o")
                for j in range(nt):
                    nc.tensor.matmul(
                        o_ps[:],
                        lhsT=g2w[:, j, :],
                        rhs=oh[:, j, :],
                        start=(j == 0),
                        stop=(j == nt - 1),
                        skip_group_check=True,
                    )
                o_sb = epi.tile([CLSP, P], F32, tag="osb")
                nc.scalar.activation(
                    o_sb[:], o_ps[:], mybir.ActivationFunctionType.Copy
                )
                nc.sync.dma_start(out=houtT[:, k * P : (k + 1) * P], in_=o_sb[:])
    nc.compile()
    _split_sync_waits(nc)
    return nc


def kernel(
    x,
    edge_index,
    W1,
    att_src1,
    att_dst1,
    b1,
    W2,
    att_src2,
    att_dst2,
    b2,
    _trace=False,
    _tmpdirs=None,
):
    x = np.asarray(x, dtype=np.float32)
    edge_index = np.asarray(edge_index).astype(np.int64)
    W1 = np.asarray(W1, dtype=np.float32)
    att_src1 = np.asarray(att_src1, dtype=np.float32)
    att_dst1 = np.asarray(att_dst1, dtype=np.float32)
    b1 = np.asarray(b1, dtype=np.float32)
    W2 = np.asarray(W2, dtype=np.float32)
    att_src2 = np.asarray(att_src2, dtype=np.float32)
    att_dst2 = np.asarray(att_dst2, dtype=np.float32)
    b2 = np.asarray(b2, dtype=np.float32)

    N, F_IN = x.shape
    HEADS, HID = att_src1.shape
    CLS = W2.shape[1]
    W2E = ((CLS + 2 + 15) // 16) * 16
    CLSP = ((CLS + 15) // 16) * 16

    key = (N, edge_index.shape[1], F_IN, HEADS, HID, CLS, hash(edge_index.tobytes()))
    if key in _CACHE:
        mt, ncA, ncB = _CACHE[key]
    else:
        mt = _preprocess(N, edge_index)
        ncA = _build_p1(mt, F_IN, HEADS, HID, CLS)
        ncB = _build_p2(mt, CLS)
        _CACHE[key] = (mt, ncA, ncB)

    NLOC, CH, T, S = mt.NLOC, mt.CH, mt.T, mt.S

    # ---- host: layer-1 alpha (a_s/a_d are linear in x) ----
    W1r = W1.reshape(F_IN, HEADS, HID)
    v_s = np.einsum("fhc,hc->fh", W1r, att_src1)
    v_d = np.einsum("fhc,hc->fh", W1r, att_dst1)
    a_s = x.astype(np.float64) @ v_s.astype(np.float64)
    a_d = x.astype(np.float64) @ v_d.astype(np.float64)
    z1 = _leaky(a_s[mt.src_s] + a_d[mt.dst_s])
    alpha1 = _seg_softmax(z1, mt.dst_s, N)

    aexp = _alpha_plane_pairs(mt, alpha1)  # [C,128,T,H,2] bf16
    iota = np.tile(np.arange(P, dtype=np.float32)[None, :], (P, 1)).astype(BF)
    dexp = np.ascontiguousarray(
        np.repeat(mt.dpos_plane[..., None], 2, axis=-1)
    ).astype(BF)  # [C,128,T,2]

    # xgT tiles: [C, 128 feat, S] bf16 = x.T columns at slot srcs
    xT16 = np.ascontiguousarray(x.astype(BF).T)  # [F_IN, N]
    w1b = W1.astype(BF)
    v_s2 = (W2 @ att_src2[0]).astype(np.float32)
    v_d2 = (W2 @ att_dst2[0]).astype(np.float32)
    w2e = np.zeros((HEADS * HID, W2E), np.float32)
    w2e[:, 0] = v_s2
    w2e[:, 1] = v_d2
    w2e[:, 2 : 2 + CLS] = W2
    w2eb = w2e.astype(BF)
    b1col = b1.reshape(P, 1).astype(np.float32)

    in_maps_a = []
    for c in range(N_CORES):
        xg = np.ascontiguousarray(xT16[:, mt.slot_src[c]])  # [128, S]
        in_maps_a.append(
            {
                "xg": xg,
                "aexp": np.ascontiguousarray(aexp[c].reshape(P, -1)),
                "dexp": np.ascontiguousarray(dexp[c].reshape(P, -1)),
                "iota": iota,
                "w1": w1b,
                "w2e": w2eb,
                "b1col": b1col,
            }
        )

    tds = _tmpdirs or [None, None]
    resA = run_bass_kernel_spmd(
        ncA, in_maps_a, list(range(N_CORES)), trace=_trace, tmpdir=tds[0]
    )

    # host: assemble tab2 + a_s2/a_d2, compute alpha2
    tab2 = np.zeros((N, CLSP), BF)
    asd = np.zeros((N, 2), np.float64)
    for c in range(N_CORES):
        hT = np.asarray(resA.results[c]["houtT"])  # [W2E, CH*P] bf16
        tab2[c * NLOC : (c + 1) * NLOC, :CLS] = hT[2 : 2 + CLS, :NLOC].T
        asd[c * NLOC : (c + 1) * NLOC] = np.asarray(
            resA.results[c]["asdT"], np.float64
        )[:, :NLOC].T

    z2 = _leaky(asd[mt.src_s, 0] + asd[mt.dst_s, 1])[:, None]
    alpha2 = _seg_softmax(z2, mt.dst_s, N)
    a2exp = _alpha_plane_pairs(mt, alpha2)  # [C,128,T,1,2]

    in_maps_b = []
    for c in range(N_CORES):
        g2 = tab2[mt.slot_src[c]]  # [S, CLSP] bf16
        g2 = np.ascontiguousarray(
            g2.reshape(T, P, CLSP).transpose(1, 0, 2).reshape(P, T * CLSP)
        )
        in_maps_b.append(
            {
                "g2": g2,
                "aexp": np.ascontiguousarray(a2exp[c].reshape(P, -1)),
                "dexp": np.ascontiguousarray(dexp[c].reshape(P, -1)),
                "iota": iota,
            }
        )

    resB = run_bass_kernel_spmd(
        ncB, in_maps_b, list(range(N_CORES)), trace=_trace, tmpdir=tds[1]
    )

    out = np.zeros((N, CLS), np.float32)
    for c in range(N_CORES):
        out[c * NLOC : (c + 1) * NLOC] = np.asarray(
            resB.results[c]["houtT"], np.float32
        )[:CLS, :NLOC].T
    out += b2[None, :]

    kernel._last = (resA, resB)
    return out


# revision 29
# speedup vs baseline: 1.2579x; 1.0235x over previous
"""GAT (2-layer, PyG-style) on 8 Trainium2 NeuronCores.

Strategy (v2 — host-staged gather, two collective-free device programs)
----------------------------------------------------------------------
- Nodes are sharded across the 8 cores by dst (N/8 rows each). Edges are
  sorted by dst and packed into 128-dst windows; each window's edges are
  padded to 128-edge tiles (slot layout identical on all cores; per-core
  counts only differ in the padding).
- The host stages the per-edge operand tiles (the "gather"):
    program P1 gets xgT tiles  — x[src_e] columns, [128 feat x 128 edge],
    program P2 gets g2 tiles   — tab2[src_e] rows, [128 edge x CLS],
  and the per-slot alpha/dst-position planes. Attention coefficients are
  computed on the host exactly as in the v1 kernel (layer-1 alpha is a
  pure function of the inputs; layer-2 alpha comes from per-node a_s2 /
  a_d2 scalars returned by P1).
- Program P1, per 128-edge tile:
    g   = xgT_tile^T @ W1            (PE, psum)
    g4  = copy psum->sbuf bf16       (ACT, batched over 4 tiles)
    gw  = g4 * alpha1                (DVE tensor_tensor, pair-expanded
                                      alpha plane to hit the 2x mode)
    oh  = (iota == dpos)             (DVE tensor_scalar, 4x mode)
    aggT += gw^T @ oh                (PE, psum accum per 128-dst window)
  and per window: h1T = Relu(aggT + b1) (ACT, bias per partition), then
  tab2T = [v_s2 | v_d2 | W2]^T @ h1T (PE) which is written out: rows
  0, 1 are a_s2 / a_d2, rows 2..CLS+1 are the layer-2 table.
- Program P2, per tile: alpha-one-hot via one fused tensor_scalar
  ((iota == dpos) * alpha2), then out[d,:] += ahot^T @ g2_tile (PE).
  Host adds b2 and reassembles the full output.

All engine work stays on device; the host does input marshalling
(edge-index bookkeeping, fancy-indexed tile staging) and the softmax
normalization of the attention logits, as in v1.

Self-contained: all shapes/structure are derived from the actual inputs.
"""

import numpy as np
import ml_dtypes

import bass_rust
import concourse.bass as bass
import concourse.bacc as bacc
import concourse.mybir as mybir
from concourse.bass_utils import run_bass_kernel_spmd
from concourse.tile import TileContext, ScopedClock

# ----------------------------------------------------------------------------
# Workaround: this walrus build rejects >1 sync wait on a CTRL op, but the
# stock TileContext tail drain carries one wait per live proc. Split them
# across nofuse NOPs (one wait each).
# ----------------------------------------------------------------------------


def _patched_drain_and_barrier(self, tick_clock, wait_clock):
    nc = self.nc
    probe = nc.sync.nop(nofuse=True, hint="tail_drain_waits")
    wait_clock.add_sem_waits(probe.ins, ScopedClock({None: tick_clock.global_clock}))
    si = probe.ins.sync_info
    waits = list(si.on_wait) if si is not None else []
    if len(waits) > 1:
        probe.ins.sync_info = bass_rust.SyncInfo(on_wait=waits[:1], on_update=[])
        for i in range(1, len(waits)):
            n = nc.sync.nop(nofuse=True, hint=f"tail_drain_waits_{i}")
            n.ins.sync_info = bass_rust.SyncInfo(on_wait=waits[i : i + 1], on_update=[])
    nc.sync.drain()
    nc.all_engine_barrier()
    assert self.sems is not None
    popped = nc._tile_sem_poison_stack.pop()
    assert popped is self._sem_poison
    nc.clear_and_free_semaphores(list(self.sems.allocated().values()))
    nc.all_engine_barrier()


TileContext._drain_and_barrier = _patched_drain_and_barrier

MAX_WAITS = 1  # this walrus build rejects instructions with more sync waits


def _split_sync_waits(nc, max_waits=MAX_WAITS):
    """Hoist excess per-instruction sync waits onto standalone nofuse NOPs
    placed immediately before the instruction (same engine)."""
    n_new = 0
    for bbname, bassbb in list(nc._state.bb_map.items()):
        bb = bassbb.bb
        insts = list(bb.instructions)
        out = []
        changed = False
        for inst in insts:
            si = inst.sync_info
            if si is not None and len(si.on_wait) > max_waits:
                waits = list(si.on_wait)
                extra = waits[:-max_waits]
                for j in range(0, len(extra), max_waits):
                    nop = mybir.InstNoOp(
                        name=f"{inst.name}-w{n_new}",
                        engine=inst.engine,
                        bass_nofuse=True,
                        sync_info=bass_rust.SyncInfo(
                            on_wait=extra[j : j + max_waits], on_update=[]
                        ),
                    )
                    n_new += 1
                    nc.register_instruction(nop, overwrite=True)
                    out.append(nop)
                inst.sync_info = bass_rust.SyncInfo(
                    on_wait=waits[-max_waits:], on_update=list(si.on_update)
                )
                changed = True
            out.append(inst)
        if changed:
            bb.instructions = out
    return n_new

# ----------------------------------------------------------------------------

P = 128
N_CORES = 8
NEG_SLOPE = 0.2
BATCH = 8  # tiles per psum batch in P1 (psum tile spans 2 banks)
SLAB = 2  # chunks per input DMA slab in P1

F32 = mybir.dt.float32
BF16 = mybir.dt.bfloat16
BF = ml_dtypes.bfloat16

_CACHE = {}


def _leaky(z):
    return np.where(z > 0, z, NEG_SLOPE * z)


def _seg_softmax(z, dst, n):
    """Exact segment softmax over sorted dst (every dst has >=1 edge)."""
    starts = np.searchsorted(dst, np.arange(n))
    m = np.maximum.reduceat(z, starts, axis=0)
    w = np.exp(z - m[dst])
    den = np.add.reduceat(w, starts, axis=0)
    return w / den[dst]


class _Meta:
    pass


def _preprocess(N, edge_index):
    """Sort edges by dst, shard by dst range, build the static window/tile
    slot structure shared by both device programs (identical on all cores;
    per-core data differs only in padding)."""
    mt = _Meta()
    assert N % N_CORES == 0
    NLOC = N // N_CORES
    CH = (NLOC + P - 1) // P
    mt.N, mt.NLOC, mt.CH = N, NLOC, CH
    mt.SH_PAD = CH * P

    src = np.concatenate([edge_index[0], np.arange(N, dtype=np.int64)])
    dst = np.concatenate([edge_index[1], np.arange(N, dtype=np.int64)])
    order = np.argsort(dst, kind="stable")
    mt.src_s, mt.dst_s = src[order], dst[order]
    E = src.shape[0]
    mt.E = E

    # per-(core, window) edge ranges
    cnt = np.zeros((N_CORES, CH), dtype=np.int64)
    rng = np.zeros((N_CORES, CH, 2), dtype=np.int64)
    for c in range(N_CORES):
        for k in range(CH):
            d0 = c * NLOC + k * P
            d1 = min(c * NLOC + min((k + 1) * P, NLOC), N)
            s = np.searchsorted(mt.dst_s, d0)
            e = np.searchsorted(mt.dst_s, d1)
            rng[c, k] = (s, e)
            cnt[c, k] = e - s

    tiles_k = ((cnt.max(axis=0) + P - 1) // P).astype(np.int64)  # per window
    mt.tiles_k = tiles_k
    mt.T = int(tiles_k.sum())
    mt.S = mt.T * P
    mt.tile_off = np.concatenate([[0], np.cumsum(tiles_k)])[:-1]  # tile idx of win k

    # per-core slot arrays
    slot_src = np.zeros((N_CORES, mt.S), dtype=np.int64)
    slot_eid = np.full((N_CORES, mt.S), -1, dtype=np.int64)
    slot_dpos = np.full((N_CORES, mt.S), -1.0, dtype=np.float64)
    for c in range(N_CORES):
        for k in range(CH):
            s, e = rng[c, k]
            ne = e - s
            s0 = int(mt.tile_off[k]) * P
            slot_src[c, s0 : s0 + ne] = mt.src_s[s:e]
            slot_eid[c, s0 : s0 + ne] = np.arange(s, e)
            slot_dpos[c, s0 : s0 + ne] = mt.dst_s[s:e] - (c * NLOC + k * P)
    mt.slot_src = slot_src
    mt.slot_eid = slot_eid

    # dpos plane [C, 128, T] f32: slot j -> [j%128, j//128]
    mt.dpos_plane = np.ascontiguousarray(
        slot_dpos.reshape(N_CORES, mt.T, P).transpose(0, 2, 1)
    ).astype(np.float32)
    return mt


def _alpha_plane_pairs(mt, alpha):
    """alpha [E, H] (dst-sorted edge order) -> [C, 128, T, H, 2] bf16 slot
    planes with each value duplicated in pairs (so the DVE 2x mode applies)."""
    H = alpha.shape[1]
    eid = mt.slot_eid
    valid = eid >= 0
    vals = np.zeros((N_CORES, mt.S, H), dtype=np.float32)
    vals[valid] = alpha[eid[valid]].astype(np.float32)
    out = vals.reshape(N_CORES, mt.T, P, H).transpose(0, 2, 1, 3)  # [C,128,T,H]
    out = np.repeat(out[..., None], 2, axis=-1)  # [C,128,T,H,2]
    return np.ascontiguousarray(out).astype(BF)


def _onehot_batch(nc, oh_sb, iota_sb, dexp_sb, t0, nt):
    """oh[:, j, d] = (iota[d] == dpos[:, t0+j]) for j in [0, nt), one DVE op.
    All operands viewed with a trailing [1,2] pair dim so the 2x mode kicks
    in (dexp is the pair-duplicated dpos plane)."""
    d_ap = dexp_sb[:, t0 : t0 + nt, :]
    d_bc = bass.AP(
        d_ap.tensor,
        d_ap.offset,
        [list(d_ap.ap[0]), [2, nt], [0, P // 2], [1, 2]],
    )
    oh_ap = oh_sb[:, :nt, :]
    oh_v = bass.AP(
        oh_ap.tensor,
        oh_ap.offset,
        [list(oh_ap.ap[0]), [P, nt], [2, P // 2], [1, 2]],
    )
    io_ap = iota_sb[:]
    io_bc = bass.AP(
        io_ap.tensor,
        io_ap.offset,
        [list(io_ap.ap[0]), [0, nt], [2, P // 2], [1, 2]],
    )
    nc.vector.tensor_tensor(
        out=oh_v, in0=io_bc, in1=d_bc, op=mybir.AluOpType.is_equal
    )


def _build_p1(mt, F_IN, HEADS, HID, CLS):
    """Layer-1 program: xgT tiles -> h1 windows -> tab2T/asd output."""
    F_HID = HEADS * HID
    CH, T, S = mt.CH, mt.T, mt.S
    W2E = ((CLS + 2 + 15) // 16) * 16  # padded [v_s2 | v_d2 | W2] columns
    assert F_IN == P and F_HID == P

    nc = bacc.Bacc("TRN2", target_bir_lowering=False, debug=False, num_devices=N_CORES)
    xg_in = nc.declare_dram_parameter("xg", [P, S], BF16, isOutput=False)
    aexp_in = nc.declare_dram_parameter("aexp", [P, T * HEADS * 2], BF16, isOutput=False)
    dexp_in = nc.declare_dram_parameter("dexp", [P, T * 2], BF16, isOutput=False)
    iota_in = nc.declare_dram_parameter("iota", [P, P], BF16, isOutput=False)
    w1_in = nc.declare_dram_parameter("w1", [P, F_HID], BF16, isOutput=False)
    w2e_in = nc.declare_dram_parameter("w2e", [P, W2E], BF16, isOutput=False)
    b1_in = nc.declare_dram_parameter("b1col", [P, 1], F32, isOutput=False)
    houtT = nc.declare_dram_parameter("houtT", [W2E, CH * P], BF16, isOutput=True)
    asdT = nc.declare_dram_parameter("asdT", [2, CH * P], F32, isOutput=True)

    # streamed-xg slab boundaries: ~17 DMAs, 4 rotating SBUF slots; the first
    # slabs are small so the first chunk's data lands quickly.
    slab_bounds = [0]
    tgt_full = (T + 15) // 16
    for k in range(CH):
        t_end = int(mt.tile_off[k]) + int(mt.tiles_k[k])
        n_so_far = len(slab_bounds) - 1
        tgt = tgt_full // 4 if n_so_far < 2 else (tgt_full // 2 if n_so_far < 3 else tgt_full)
        if (t_end - slab_bounds[-1] >= tgt or k == CH - 1) and t_end > slab_bounds[-1]:
            slab_bounds.append(t_end)
    chunk_slab = {}
    for k in range(CH):
        t0 = int(mt.tile_off[k])
        chunk_slab[k] = sum(1 for b in slab_bounds[1:-1] if b <= t0)

    with TileContext(nc) as tc:
        with (
            tc.tile_pool(name="res", bufs=1) as res,
            tc.tile_pool(name="g4p", bufs=3) as g4p,
            tc.tile_pool(name="gwp", bufs=3) as gwp,
            tc.tile_pool(name="ohp", bufs=3) as ohp,
            tc.tile_pool(name="epi", bufs=3) as epi,
            tc.tile_pool(name="psg", bufs=2, space="PSUM") as psg,
            tc.tile_pool(name="psa", bufs=2, space="PSUM") as psa,
            tc.tile_pool(name="pst", bufs=2, space="PSUM") as pst,
        ):
            # residents on the ACT HWDGE ring; data slabs on the SP ring, so
            # the first slab isn't queued behind the big alpha plane.
            w1_sb = res.tile([P, F_HID], BF16)
            nc.scalar.dma_start(out=w1_sb[:], in_=w1_in[:])
            iota_sb = res.tile([P, P], BF16)
            nc.scalar.dma_start(out=iota_sb[:], in_=iota_in[:])
            b1_sb = res.tile([P, 1], F32)
            nc.scalar.dma_start(out=b1_sb[:], in_=b1_in[:])
            dexp_sb = res.tile([P, T, 2], BF16)
            nc.scalar.dma_start(
                out=dexp_sb[:],
                in_=dexp_in[:].rearrange("p (t two) -> p t two", two=2),
            )
            aexp_sb = res.tile([P, T, HEADS, 2], BF16)
            nc.scalar.dma_start(
                out=aexp_sb[:],
                in_=aexp_in[:].rearrange("p (t h two) -> p t h two", h=HEADS, two=2),
            )
            w2e_sb = res.tile([P, W2E], BF16)
            nc.scalar.dma_start(out=w2e_sb[:], in_=w2e_in[:])

            max_slab = max(
                slab_bounds[i + 1] - slab_bounds[i]
                for i in range(len(slab_bounds) - 1)
            )
            xg_slabs = []
            for si in range(len(slab_bounds) - 1):
                a, b = slab_bounds[si], slab_bounds[si + 1]
                sl = res.tile(
                    [P, max_slab, P], BF16, name=f"xgsl{si}", tag=f"xgsl{si % 4}"
                )[:, : b - a, :]
                nc.sync.dma_start(
                    out=sl[:],
                    in_=xg_in[:, a * P : b * P].rearrange("p (t e) -> p t e", e=P),
                )
                xg_slabs.append(sl)

            for k in range(CH):
                nt = int(mt.tiles_k[k])
                t0 = int(mt.tile_off[k])
                si = chunk_slab[k]
                xg_sb = xg_slabs[si][
                    :, t0 - slab_bounds[si] : t0 - slab_bounds[si] + nt, :
                ]
                agg_ps = psa.tile([P, P], F32, tag="agg")
                g_sb = g4p.tile([P, nt, P], BF16, tag="g")
                n_batches = (nt + BATCH - 1) // BATCH
                for b in range(n_batches):
                    j0 = b * BATCH
                    nb = min(BATCH, nt - j0)
                    g_ps = psg.tile([P, BATCH * P], F32, tag="gps")
                    for j in range(nb):
                        nc.tensor.matmul(
                            g_ps[:, j * P : (j + 1) * P],
                            lhsT=xg_sb[:, j0 + j, :],
                            rhs=w1_sb[:],
                            start=True,
                            stop=True,
                            skip_group_check=True,
                        )
                    nc.scalar.activation(
                        g_sb[:, j0 : j0 + nb, :],
                        g_ps[:, : nb * P].rearrange("p (t e) -> p t e", e=P),
                        mybir.ActivationFunctionType.Copy,
                    )
                # gw = g * alpha, split at the psum-batch boundary so the
                # aggregation matmuls start before the last copy lands
                gw = gwp.tile([P, nt, P], BF16, tag="gw")
                def gview(tile, lo, n):
                    sl = tile[:, lo : lo + n, :]
                    return bass.AP(
                        sl.tensor,
                        sl.offset,
                        [list(sl.ap[0]), [HID, n * HEADS], [2, HID // 2], [1, 2]],
                    )
                mult_splits = [(0, min(2 * BATCH, nt))]
                if nt > 2 * BATCH:
                    mult_splits.append((2 * BATCH, nt - 2 * BATCH))
                for lo, n in mult_splits:
                    a_ap = aexp_sb[:, t0 + lo : t0 + lo + n, :, :]
                    a_bc = bass.AP(
                        a_ap.tensor,
                        a_ap.offset,
                        [list(a_ap.ap[0]), [2, n * HEADS], [0, HID // 2], [1, 2]],
                    )
                    nc.vector.tensor_tensor(
                        out=gview(gw, lo, n),
                        in0=gview(g_sb, lo, n),
                        in1=a_bc,
                        op=mybir.AluOpType.mult,
                    )
                oh = ohp.tile([P, nt, P], BF16, tag="oh")
                _onehot_batch(nc, oh, iota_sb, dexp_sb, t0, nt)
                for j in range(nt):
                    nc.tensor.matmul(
                        agg_ps[:],
                        lhsT=gw[:, j, :],
                        rhs=oh[:, j, :],
                        start=(j == 0),
                        stop=(j == nt - 1),
                        skip_group_check=True,
                    )
                # epilogue: h1T = relu(aggT + b1), tab2T = W2e^T @ h1T
                h1T = epi.tile([P, P], BF16, tag="h1T")
                nc.scalar.activation(
                    h1T[:],
                    agg_ps[:],
                    mybir.ActivationFunctionType.Relu,
                    bias=b1_sb[:, 0:1],
                )
                t2_ps = pst.tile([W2E, P], F32, tag="t2")
                nc.tensor.matmul(
                    t2_ps[:],
                    lhsT=w2e_sb[:],
                    rhs=h1T[:],
                    start=True,
                    stop=True,
                    skip_group_check=True,
                )
                t2_sb = epi.tile([W2E, P], BF16, tag="t2sb")
                nc.vector.tensor_copy(out=t2_sb[:], in_=t2_ps[:])
                asd_sb = epi.tile([2, P], F32, tag="asd")
                nc.vector.tensor_copy(out=asd_sb[:], in_=t2_ps[0:2, :])
                nc.sync.dma_start(out=houtT[:, k * P : (k + 1) * P], in_=t2_sb[:])
                nc.sync.dma_start(out=asdT[:, k * P : (k + 1) * P], in_=asd_sb[:])
    nc.compile()
    _split_sync_waits(nc)
    return nc


def _build_p2(mt, CLS):
    """Layer-2 program: g2 tiles * alpha2 aggregated per window (transposed
    matmul so the PE streams 128 columns and leaves the cold p-state)."""
    CH, T, S = mt.CH, mt.T, mt.S
    CLSP = ((CLS + 15) // 16) * 16

    nc = bacc.Bacc("TRN2", target_bir_lowering=False, debug=False, num_devices=N_CORES)
    g2_in = nc.declare_dram_parameter("g2", [P, T * CLSP], BF16, isOutput=False)
    aexp_in = nc.declare_dram_parameter("aexp", [P, T * 2], BF16, isOutput=False)
    dexp_in = nc.declare_dram_parameter("dexp", [P, T * 2], BF16, isOutput=False)
    iota_in = nc.declare_dram_parameter("iota", [P, P], BF16, isOutput=False)
    houtT = nc.declare_dram_parameter("houtT", [CLSP, CH * P], F32, isOutput=True)

    # resident-g2 slab boundaries: ~4 big DMAs aligned to chunk starts
    slab_bounds = [0]
    tgt = (T + 7) // 8
    for k in range(CH):
        t_end = int(mt.tile_off[k]) + int(mt.tiles_k[k])
        if (t_end - slab_bounds[-1] >= tgt or k == CH - 1) and t_end > slab_bounds[-1]:
            slab_bounds.append(t_end)
    chunk_slab = {}
    for k in range(CH):
        t0 = int(mt.tile_off[k])
        chunk_slab[k] = sum(1 for b in slab_bounds[1:-1] if b <= t0)

    with TileContext(nc) as tc:
        with (
            tc.tile_pool(name="res", bufs=1) as res,
            tc.tile_pool(name="gwp", bufs=3) as gwp,
            tc.tile_pool(name="ohp", bufs=3) as ohp,
            tc.tile_pool(name="epi", bufs=3) as epi,
            tc.tile_pool(name="pso", bufs=2, space="PSUM") as pso,
        ):
            iota_sb = res.tile([P, P], BF16)
            nc.scalar.dma_start(out=iota_sb[:], in_=iota_in[:])
            dexp_sb = res.tile([P, T, 2], BF16)
            nc.scalar.dma_start(
                out=dexp_sb[:],
                in_=dexp_in[:].rearrange("p (t two) -> p t two", two=2),
            )
            aexp_sb = res.tile([P, T, 2], BF16)
            nc.scalar.dma_start(
                out=aexp_sb[:],
                in_=aexp_in[:].rearrange("p (t two) -> p t two", two=2),
            )
            g2_slabs = []
            for si in range(len(slab_bounds) - 1):
                a, b = slab_bounds[si], slab_bounds[si + 1]
                sl = res.tile([P, b - a, CLSP], BF16, name=f"g2sl{si}", tag=f"g2sl{si}")
                nc.sync.dma_start(
                    out=sl[:],
                    in_=g2_in[:, a * CLSP : b * CLSP].rearrange(
                        "p (t e) -> p t e", e=CLSP
                    ),
                )
                g2_slabs.append(sl)

            for k in range(CH):
                nt = int(mt.tiles_k[k])
                t0 = int(mt.tile_off[k])
                si = chunk_slab[k]
                g2_sb = g2_slabs[si][:, t0 - slab_bounds[si] : t0 - slab_bounds[si] + nt, :]
                # g2w = g2 * alpha2 (one DVE op per chunk, 2x mode)
                g2w = gwp.tile([P, nt, CLSP], BF16, tag="g2w")
                a_ap = aexp_sb[:, t0 : t0 + nt, :]
                a_bc = bass.AP(
                    a_ap.tensor,
                    a_ap.offset,
                    [list(a_ap.ap[0]), [2, nt], [0, CLSP // 2], [1, 2]],
                )
                gview = lambda tile: bass.AP(
                    tile.tensor,
                    tile.offset,
                    [list(tile.ap[0]), [CLSP, nt], [2, CLSP // 2], [1, 2]],
                )
                nc.vector.tensor_tensor(
                    out=gview(g2w[:, :, :]),
                    in0=gview(g2_sb[:, :, :]),
                    in1=a_bc,
                    op=mybir.AluOpType.mult,
                )
                oh = ohp.tile([P, nt, P], BF16, tag="oh")
                _onehot_batch(nc, oh, iota_sb, dexp_sb, t0, nt)
                o_ps = pso.tile([CLSP, P], F32, tag="o")
                for j in range(nt):
                    nc.tensor.matmul(
                        o_ps[:],
                        lhsT=g2w[:, j, :],
                        rhs=oh[:, j, :],
                        start=(j == 0),
                        stop=(j == nt - 1),
                        skip_group_check=True,
                    )
                o_sb = epi.tile([CLSP, P], F32, tag="osb")
                nc.scalar.activation(
                    o_sb[:], o_ps[:], mybir.ActivationFunctionType.Copy
                )
                nc.sync.dma_start(out=houtT[:, k * P : (k + 1) * P], in_=o_sb[:])
    nc.compile()
    _split_sync_waits(nc)
    return nc


def kernel(
    x,
    edge_index,
    W1,
    att_src1,
    att_dst1,
    b1,
    W2,
    att_src2,
    att_dst2,
    b2,
    _trace=False,
    _tmpdirs=None,
):
    x = np.asarray(x, dtype=np.float32)
    edge_index = np.asarray(edge_index).astype(np.int64)
    W1 = np.asarray(W1, dtype=np.float32)
    att_src1 = np.asarray(att_src1, dtype=np.float32)
    att_dst1 = np.asarray(att_dst1, dtype=np.float32)
    b1 = np.asarray(b1, dtype=np.float32)
    W2 = np.asarray(W2, dtype=np.float32)
    att_src2 = np.asarray(att_src2, dtype=np.float32)
    att_dst2 = np.asarray(att_dst2, dtype=np.float32)
    b2 = np.asarray(b2, dtype=np.float32)

    N, F_IN = x.shape
    HEADS, HID = att_src1.shape
    CLS = W2.shape[1]
    W2E = ((CLS + 2 + 15) // 16) * 16
    CLSP = ((CLS + 15) // 16) * 16

    key = (N, edge_index.shape[1], F_IN, HEADS, HID, CLS, hash(edge_index.tobytes()))
    if key in _CACHE:
        mt, ncA, ncB = _CACHE[key]
    else:
        mt = _preprocess(N, edge_index)
        ncA = _build_p1(mt, F_IN, HEADS, HID, CLS)
        ncB = _build_p2(mt, CLS)
        _CACHE[key] = (mt, ncA, ncB)

    NLOC, CH, T, S = mt.NLOC, mt.CH, mt.T, mt.S

    # ---- host: layer-1 alpha (a_s/a_d are linear in x) ----
    W1r = W1.reshape(F_IN, HEADS, HID)
    v_s = np.einsum("fhc,hc->fh", W1r, att_src1)
    v_d = np.einsum("fhc,hc->fh", W1r, att_dst1)
    a_s = x.astype(np.float64) @ v_s.astype(np.float64)
    a_d = x.astype(np.float64) @ v_d.astype(np.float64)
    z1 = _leaky(a_s[mt.src_s] + a_d[mt.dst_s])
    alpha1 = _seg_softmax(z1, mt.dst_s, N)

    aexp = _alpha_plane_pairs(mt, alpha1)  # [C,128,T,H,2] bf16
    iota = np.tile(np.arange(P, dtype=np.float32)[None, :], (P, 1)).astype(BF)
    dexp = np.ascontiguousarray(
        np.repeat(mt.dpos_plane[..., None], 2, axis=-1)
    ).astype(BF)  # [C,128,T,2]

    # xgT tiles: [C, 128 feat, S] bf16 = x.T columns at slot srcs
    xT16 = np.ascontiguousarray(x.astype(BF).T)  # [F_IN, N]
    w1b = W1.astype(BF)
    v_s2 = (W2 @ att_src2[0]).astype(np.float32)
    v_d2 = (W2 @ att_dst2[0]).astype(np.float32)
    w2e = np.zeros((HEADS * HID, W2E), np.float32)
    w2e[:, 0] = v_s2
    w2e[:, 1] = v_d2
    w2e[:, 2 : 2 + CLS] = W2
    w2eb = w2e.astype(BF)
    b1col = b1.reshape(P, 1).astype(np.float32)

    in_maps_a = []
    for c in range(N_CORES):
        xg = np.ascontiguousarray(xT16[:, mt.slot_src[c]])  # [128, S]
        in_maps_a.append(
            {
                "xg": xg,
                "aexp": np.ascontiguousarray(aexp[c].reshape(P, -1)),
                "dexp": np.ascontiguousarray(dexp[c].reshape(P, -1)),
                "iota": iota,
                "w1": w1b,
                "w2e": w2eb,
                "b1col": b1col,
            }
        )

    tds = _tmpdirs or [None, None]
    resA = run_bass_kernel_spmd(
        ncA, in_maps_a, list(range(N_CORES)), trace=_trace, tmpdir=tds[0]
    )

    # host: assemble tab2 + a_s2/a_d2, compute alpha2
    tab2 = np.zeros((N, CLSP), BF)
    asd = np.zeros((N, 2), np.float64)
    for c in range(N_CORES):
        hT = np.asarray(resA.results[c]["houtT"])  # [W2E, CH*P] bf16
        tab2[c * NLOC : (c + 1) * NLOC, :CLS] = hT[2 : 2 + CLS, :NLOC].T
        asd[c * NLOC : (c + 1) * NLOC] = np.asarray(
            resA.results[c]["asdT"], np.float64
        )[:, :NLOC].T

    z2 = _leaky(asd[mt.src_s, 0] + asd[mt.dst_s, 1])[:, None]
    alpha2 = _seg_softmax(z2, mt.dst_s, N)
    a2exp = _alpha_plane_pairs(mt, alpha2)  # [C,128,T,1,2]

    in_maps_b = []
    for c in range(N_CORES):
        g2 = tab2[mt.slot_src[c]]  # [S, CLSP] bf16
        g2 = np.ascontiguousarray(
            g2.reshape(T, P, CLSP).transpose(1, 0, 2).reshape(P, T * CLSP)
        )
        in_maps_b.append(
            {
                "g2": g2,
                "aexp": np.ascontiguousarray(a2exp[c].reshape(P, -1)),
                "dexp": np.ascontiguousarray(dexp[c].reshape(P, -1)),
                "iota": iota,
            }
        )

    resB = run_bass_kernel_spmd(
        ncB, in_maps_b, list(range(N_CORES)), trace=_trace, tmpdir=tds[1]
    )

    out = np.zeros((N, CLS), np.float32)
    for c in range(N_CORES):
        out[c * NLOC : (c + 1) * NLOC] = np.asarray(
            resB.results[c]["houtT"], np.float32
        )[:CLS, :NLOC].T
    out += b2[None, :]

    kernel._last = (resA, resB)
    return out


# revision 33
# speedup vs baseline: 1.2990x; 1.0327x over previous
"""GAT (2-layer, PyG-style) on 8 Trainium2 NeuronCores.

Strategy (v2 — host-staged gather, two collective-free device programs)
----------------------------------------------------------------------
- Nodes are sharded across the 8 cores by dst (N/8 rows each). Edges are
  sorted by dst and packed into 128-dst windows; each window's edges are
  padded to 128-edge tiles (slot layout identical on all cores; per-core
  counts only differ in the padding).
- The host stages the per-edge operand tiles (the "gather"):
    program P1 gets xgT tiles  — x[src_e] columns, [128 feat x 128 edge],
    program P2 gets g2 tiles   — tab2[src_e] rows, [128 edge x CLS],
  and the per-slot alpha/dst-position planes. Attention coefficients are
  computed on the host exactly as in the v1 kernel (layer-1 alpha is a
  pure function of the inputs; layer-2 alpha comes from per-node a_s2 /
  a_d2 scalars returned by P1).
- Program P1, per 128-edge tile:
    g   = xgT_tile^T @ W1            (PE, psum)
    g4  = copy psum->sbuf bf16       (ACT, batched over 4 tiles)
    gw  = g4 * alpha1                (DVE tensor_tensor, pair-expanded
                                      alpha plane to hit the 2x mode)
    oh  = (iota == dpos)             (DVE tensor_scalar, 4x mode)
    aggT += gw^T @ oh                (PE, psum accum per 128-dst window)
  and per window: h1T = Relu(aggT + b1) (ACT, bias per partition), then
  tab2T = [v_s2 | v_d2 | W2]^T @ h1T (PE) which is written out: rows
  0, 1 are a_s2 / a_d2, rows 2..CLS+1 are the layer-2 table.
- Program P2, per tile: alpha-one-hot via one fused tensor_scalar
  ((iota == dpos) * alpha2), then out[d,:] += ahot^T @ g2_tile (PE).
  Host adds b2 and reassembles the full output.

All engine work stays on device; the host does input marshalling
(edge-index bookkeeping, fancy-indexed tile staging) and the softmax
normalization of the attention logits, as in v1.

Self-contained: all shapes/structure are derived from the actual inputs.
"""

import numpy as np
import ml_dtypes

import bass_rust
import concourse.bass as bass
import concourse.bacc as bacc
import concourse.mybir as mybir
from concourse.bass_utils import run_bass_kernel_spmd
from concourse.tile import TileContext, ScopedClock

# ----------------------------------------------------------------------------
# Workaround: this walrus build rejects >1 sync wait on a CTRL op, but the
# stock TileContext tail drain carries one wait per live proc. Split them
# across nofuse NOPs (one wait each).
# ----------------------------------------------------------------------------


def _patched_drain_and_barrier(self, tick_clock, wait_clock):
    nc = self.nc
    probe = nc.sync.nop(nofuse=True, hint="tail_drain_waits")
    wait_clock.add_sem_waits(probe.ins, ScopedClock({None: tick_clock.global_clock}))
    si = probe.ins.sync_info
    waits = list(si.on_wait) if si is not None else []
    if len(waits) > 1:
        probe.ins.sync_info = bass_rust.SyncInfo(on_wait=waits[:1], on_update=[])
        for i in range(1, len(waits)):
            n = nc.sync.nop(nofuse=True, hint=f"tail_drain_waits_{i}")
            n.ins.sync_info = bass_rust.SyncInfo(on_wait=waits[i : i + 1], on_update=[])
    nc.sync.drain()
    nc.all_engine_barrier()
    assert self.sems is not None
    popped = nc._tile_sem_poison_stack.pop()
    assert popped is self._sem_poison
    nc.clear_and_free_semaphores(list(self.sems.allocated().values()))
    nc.all_engine_barrier()


TileContext._drain_and_barrier = _patched_drain_and_barrier

MAX_WAITS = 1  # this walrus build rejects instructions with more sync waits


def _split_sync_waits(nc, max_waits=MAX_WAITS):
    """Hoist excess per-instruction sync waits onto standalone nofuse NOPs
    placed immediately before the instruction (same engine)."""
    n_new = 0
    for bbname, bassbb in list(nc._state.bb_map.items()):
        bb = bassbb.bb
        insts = list(bb.instructions)
        out = []
        changed = False
        for inst in insts:
            si = inst.sync_info
            if si is not None and len(si.on_wait) > max_waits:
                waits = list(si.on_wait)
                extra = waits[:-max_waits]
                for j in range(0, len(extra), max_waits):
                    nop = mybir.InstNoOp(
                        name=f"{inst.name}-w{n_new}",
                        engine=inst.engine,
                        bass_nofuse=True,
                        sync_info=bass_rust.SyncInfo(
                            on_wait=extra[j : j + max_waits], on_update=[]
                        ),
                    )
                    n_new += 1
                    nc.register_instruction(nop, overwrite=True)
                    out.append(nop)
                inst.sync_info = bass_rust.SyncInfo(
                    on_wait=waits[-max_waits:], on_update=list(si.on_update)
                )
                changed = True
            out.append(inst)
        if changed:
            bb.instructions = out
    return n_new

# ----------------------------------------------------------------------------

P = 128
N_CORES = 8
NEG_SLOPE = 0.2
BATCH = 8  # tiles per psum batch in P1 (psum tile spans 2 banks)
SLAB = 2  # chunks per input DMA slab in P1

F32 = mybir.dt.float32
BF16 = mybir.dt.bfloat16
BF = ml_dtypes.bfloat16

_CACHE = {}


def _leaky(z):
    return np.where(z > 0, z, NEG_SLOPE * z)


def _seg_softmax(z, dst, n):
    """Exact segment softmax over sorted dst (every dst has >=1 edge)."""
    starts = np.searchsorted(dst, np.arange(n))
    m = np.maximum.reduceat(z, starts, axis=0)
    w = np.exp(z - m[dst])
    den = np.add.reduceat(w, starts, axis=0)
    return w / den[dst]


class _Meta:
    pass


def _preprocess(N, edge_index):
    """Sort edges by dst, shard by dst range, build the static window/tile
    slot structure shared by both device programs (identical on all cores;
    per-core data differs only in padding)."""
    mt = _Meta()
    assert N % N_CORES == 0
    NLOC = N // N_CORES
    CH = (NLOC + P - 1) // P
    mt.N, mt.NLOC, mt.CH = N, NLOC, CH
    mt.SH_PAD = CH * P

    src = np.concatenate([edge_index[0], np.arange(N, dtype=np.int64)])
    dst = np.concatenate([edge_index[1], np.arange(N, dtype=np.int64)])
    order = np.argsort(dst, kind="stable")
    mt.src_s, mt.dst_s = src[order], dst[order]
    E = src.shape[0]
    mt.E = E

    # per-(core, window) edge ranges
    cnt = np.zeros((N_CORES, CH), dtype=np.int64)
    rng = np.zeros((N_CORES, CH, 2), dtype=np.int64)
    for c in range(N_CORES):
        for k in range(CH):
            d0 = c * NLOC + k * P
            d1 = min(c * NLOC + min((k + 1) * P, NLOC), N)
            s = np.searchsorted(mt.dst_s, d0)
            e = np.searchsorted(mt.dst_s, d1)
            rng[c, k] = (s, e)
            cnt[c, k] = e - s

    tiles_k = ((cnt.max(axis=0) + P - 1) // P).astype(np.int64)  # per window
    mt.tiles_k = tiles_k
    mt.T = int(tiles_k.sum())
    mt.S = mt.T * P
    mt.tile_off = np.concatenate([[0], np.cumsum(tiles_k)])[:-1]  # tile idx of win k

    # per-core slot arrays
    slot_src = np.zeros((N_CORES, mt.S), dtype=np.int64)
    slot_eid = np.full((N_CORES, mt.S), -1, dtype=np.int64)
    slot_dpos = np.full((N_CORES, mt.S), -1.0, dtype=np.float64)
    for c in range(N_CORES):
        for k in range(CH):
            s, e = rng[c, k]
            ne = e - s
            s0 = int(mt.tile_off[k]) * P
            slot_src[c, s0 : s0 + ne] = mt.src_s[s:e]
            slot_eid[c, s0 : s0 + ne] = np.arange(s, e)
            slot_dpos[c, s0 : s0 + ne] = mt.dst_s[s:e] - (c * NLOC + k * P)
    mt.slot_src = slot_src
    mt.slot_eid = slot_eid

    # dpos plane [C, 128, T] f32: slot j -> [j%128, j//128]
    mt.dpos_plane = np.ascontiguousarray(
        slot_dpos.reshape(N_CORES, mt.T, P).transpose(0, 2, 1)
    ).astype(np.float32)
    return mt


def _alpha_plane_pairs(mt, alpha):
    """alpha [E, H] (dst-sorted edge order) -> [C, 128, T, H, 2] bf16 slot
    planes with each value duplicated in pairs (so the DVE 2x mode applies)."""
    H = alpha.shape[1]
    eid = mt.slot_eid
    valid = eid >= 0
    vals = np.zeros((N_CORES, mt.S, H), dtype=np.float32)
    vals[valid] = alpha[eid[valid]].astype(np.float32)
    out = vals.reshape(N_CORES, mt.T, P, H).transpose(0, 2, 1, 3)  # [C,128,T,H]
    out = np.repeat(out[..., None], 2, axis=-1)  # [C,128,T,H,2]
    return np.ascontiguousarray(out).astype(BF)


def _onehot_batch(nc, oh_sb, iota_sb, dexp_sb, t0, nt):
    """oh[:, j, d] = (iota[d] == dpos[:, t0+j]) for j in [0, nt), one DVE op.
    All operands viewed with a trailing [1,2] pair dim so the 2x mode kicks
    in (dexp is the pair-duplicated dpos plane)."""
    d_ap = dexp_sb[:, t0 : t0 + nt, :]
    d_bc = bass.AP(
        d_ap.tensor,
        d_ap.offset,
        [list(d_ap.ap[0]), [2, nt], [0, P // 2], [1, 2]],
    )
    oh_ap = oh_sb[:, :nt, :]
    oh_v = bass.AP(
        oh_ap.tensor,
        oh_ap.offset,
        [list(oh_ap.ap[0]), [P, nt], [2, P // 2], [1, 2]],
    )
    io_ap = iota_sb[:]
    io_bc = bass.AP(
        io_ap.tensor,
        io_ap.offset,
        [list(io_ap.ap[0]), [0, nt], [2, P // 2], [1, 2]],
    )
    nc.vector.tensor_tensor(
        out=oh_v, in0=io_bc, in1=d_bc, op=mybir.AluOpType.is_equal
    )


def _build_p1(mt, F_IN, HEADS, HID, CLS):
    """Layer-1 program: xgT tiles -> h1 windows -> tab2T/asd output."""
    F_HID = HEADS * HID
    CH, T, S = mt.CH, mt.T, mt.S
    W2E = ((CLS + 2 + 15) // 16) * 16  # padded [v_s2 | v_d2 | W2] columns
    assert F_IN == P and F_HID == P

    nc = bacc.Bacc("TRN2", target_bir_lowering=False, debug=False, num_devices=N_CORES)
    xg_in = nc.declare_dram_parameter("xg", [P, S], BF16, isOutput=False)
    aexp_in = nc.declare_dram_parameter("aexp", [P, T * HEADS * 2], BF16, isOutput=False)
    dexp_in = nc.declare_dram_parameter("dexp", [P, T * 2], BF16, isOutput=False)
    iota_in = nc.declare_dram_parameter("iota", [P, P], BF16, isOutput=False)
    w1_in = nc.declare_dram_parameter("w1", [P, F_HID], BF16, isOutput=False)
    w2e_in = nc.declare_dram_parameter("w2e", [P, W2E], BF16, isOutput=False)
    b1_in = nc.declare_dram_parameter("b1col", [P, 1], F32, isOutput=False)
    houtT = nc.declare_dram_parameter("houtT", [W2E, CH * P], BF16, isOutput=True)
    asdT = nc.declare_dram_parameter("asdT", [2, CH * P], F32, isOutput=True)

    # streamed-xg slab boundaries: ~17 DMAs, 4 rotating SBUF slots; the first
    # slabs are small so the first chunk's data lands quickly.
    slab_bounds = [0]
    tgt_full = (T + 15) // 16
    for k in range(CH):
        t_end = int(mt.tile_off[k]) + int(mt.tiles_k[k])
        n_so_far = len(slab_bounds) - 1
        tgt = tgt_full // 4 if n_so_far < 2 else (tgt_full // 2 if n_so_far < 3 else tgt_full)
        if (t_end - slab_bounds[-1] >= tgt or k == CH - 1) and t_end > slab_bounds[-1]:
            slab_bounds.append(t_end)
    chunk_slab = {}
    for k in range(CH):
        t0 = int(mt.tile_off[k])
        chunk_slab[k] = sum(1 for b in slab_bounds[1:-1] if b <= t0)

    with TileContext(nc) as tc:
        with (
            tc.tile_pool(name="res", bufs=1) as res,
            tc.tile_pool(name="g4p", bufs=3) as g4p,
            tc.tile_pool(name="gwp", bufs=3) as gwp,
            tc.tile_pool(name="ohp", bufs=3) as ohp,
            tc.tile_pool(name="epi", bufs=3) as epi,
            tc.tile_pool(name="psg", bufs=2, space="PSUM") as psg,
            tc.tile_pool(name="psa", bufs=2, space="PSUM") as psa,
            tc.tile_pool(name="pst", bufs=2, space="PSUM") as pst,
        ):
            # residents on the ACT HWDGE ring; data slabs on the SP ring, so
            # the first slab isn't queued behind the big alpha plane.
            w1_sb = res.tile([P, F_HID], BF16)
            nc.scalar.dma_start(out=w1_sb[:], in_=w1_in[:])
            iota_sb = res.tile([P, P], BF16)
            nc.scalar.dma_start(out=iota_sb[:], in_=iota_in[:])
            b1_sb = res.tile([P, 1], F32)
            nc.scalar.dma_start(out=b1_sb[:], in_=b1_in[:])
            dexp_sb = res.tile([P, T, 2], BF16)
            nc.scalar.dma_start(
                out=dexp_sb[:],
                in_=dexp_in[:].rearrange("p (t two) -> p t two", two=2),
            )
            aexp_sb = res.tile([P, T, HEADS, 2], BF16)
            nc.scalar.dma_start(
                out=aexp_sb[:],
                in_=aexp_in[:].rearrange("p (t h two) -> p t h two", h=HEADS, two=2),
            )
            w2e_sb = res.tile([P, W2E], BF16)
            nc.scalar.dma_start(out=w2e_sb[:], in_=w2e_in[:])

            max_slab = max(
                slab_bounds[i + 1] - slab_bounds[i]
                for i in range(len(slab_bounds) - 1)
            )
            xg_slabs = []
            for si in range(len(slab_bounds) - 1):
                a, b = slab_bounds[si], slab_bounds[si + 1]
                sl = res.tile(
                    [P, max_slab, P], BF16, name=f"xgsl{si}", tag=f"xgsl{si % 4}"
                )[:, : b - a, :]
                nc.sync.dma_start(
                    out=sl[:],
                    in_=xg_in[:, a * P : b * P].rearrange("p (t e) -> p t e", e=P),
                )
                xg_slabs.append(sl)

            for k in range(CH):
                nt = int(mt.tiles_k[k])
                t0 = int(mt.tile_off[k])
                si = chunk_slab[k]
                xg_sb = xg_slabs[si][
                    :, t0 - slab_bounds[si] : t0 - slab_bounds[si] + nt, :
                ]
                agg_ps = psa.tile([P, P], F32, tag="agg")
                g_sb = g4p.tile([P, nt, P], BF16, tag="g")
                n_batches = (nt + BATCH - 1) // BATCH
                for b in range(n_batches):
                    j0 = b * BATCH
                    nb = min(BATCH, nt - j0)
                    g_ps = psg.tile([P, BATCH * P], F32, tag="gps")
                    for j in range(nb):
                        nc.tensor.matmul(
                            g_ps[:, j * P : (j + 1) * P],
                            lhsT=xg_sb[:, j0 + j, :],
                            rhs=w1_sb[:],
                            start=True,
                            stop=True,
                            skip_group_check=True,
                        )
                    nc.scalar.activation(
                        g_sb[:, j0 : j0 + nb, :],
                        g_ps[:, : nb * P].rearrange("p (t e) -> p t e", e=P),
                        mybir.ActivationFunctionType.Copy,
                    )
                # gw = g * alpha, split at the psum-batch boundary so the
                # aggregation matmuls start before the last copy lands
                gw = gwp.tile([P, nt, P], BF16, tag="gw")
                def gview(tile, lo, n):
                    sl = tile[:, lo : lo + n, :]
                    return bass.AP(
                        sl.tensor,
                        sl.offset,
                        [list(sl.ap[0]), [HID, n * HEADS], [2, HID // 2], [1, 2]],
                    )
                mult_splits = [(0, min(2 * BATCH, nt))]
                if nt > 2 * BATCH:
                    mult_splits.append((2 * BATCH, nt - 2 * BATCH))
                for lo, n in mult_splits:
                    a_ap = aexp_sb[:, t0 + lo : t0 + lo + n, :, :]
                    a_bc = bass.AP(
                        a_ap.tensor,
                        a_ap.offset,
                        [list(a_ap.ap[0]), [2, n * HEADS], [0, HID // 2], [1, 2]],
                    )
                    nc.vector.tensor_tensor(
                        out=gview(gw, lo, n),
                        in0=gview(g_sb, lo, n),
                        in1=a_bc,
                        op=mybir.AluOpType.mult,
                    )
                oh = ohp.tile([P, nt, P], BF16, tag="oh")
                _onehot_batch(nc, oh, iota_sb, dexp_sb, t0, nt)
                for j in range(nt):
                    nc.tensor.matmul(
                        agg_ps[:],
                        lhsT=gw[:, j, :],
                        rhs=oh[:, j, :],
                        start=(j == 0),
                        stop=(j == nt - 1),
                        skip_group_check=True,
                    )
                # epilogue: h1T = relu(aggT + b1), tab2T = W2e^T @ h1T
                h1T = epi.tile([P, P], BF16, tag="h1T")
                nc.scalar.activation(
                    h1T[:],
                    agg_ps[:],
                    mybir.ActivationFunctionType.Relu,
                    bias=b1_sb[:, 0:1],
                )
                t2_ps = pst.tile([W2E, P], F32, tag="t2")
                nc.tensor.matmul(
                    t2_ps[:],
                    lhsT=w2e_sb[:],
                    rhs=h1T[:],
                    start=True,
                    stop=True,
                    skip_group_check=True,
                )
                t2_sb = epi.tile([W2E, P], BF16, tag="t2sb")
                nc.vector.tensor_copy(out=t2_sb[:], in_=t2_ps[:])
                asd_sb = epi.tile([2, P], F32, tag="asd")
                nc.vector.tensor_copy(out=asd_sb[:], in_=t2_ps[0:2, :])
                nc.sync.dma_start(out=houtT[:, k * P : (k + 1) * P], in_=t2_sb[:])
                nc.sync.dma_start(out=asdT[:, k * P : (k + 1) * P], in_=asd_sb[:])
    nc.compile()
    _split_sync_waits(nc)
    return nc


def _build_p2(mt, CLS):
    """Layer-2 program: g2 tiles * alpha2 aggregated per window (transposed
    matmul so the PE streams 128 columns and leaves the cold p-state)."""
    CH, T, S = mt.CH, mt.T, mt.S
    CLSP = ((CLS + 15) // 16) * 16

    nc = bacc.Bacc("TRN2", target_bir_lowering=False, debug=False, num_devices=N_CORES)
    g2_in = nc.declare_dram_parameter("g2", [P, T * CLSP], BF16, isOutput=False)
    aexp_in = nc.declare_dram_parameter("aexp", [P, T * 2], BF16, isOutput=False)
    dexp_in = nc.declare_dram_parameter("dexp", [P, T * 2], BF16, isOutput=False)
    iota_in = nc.declare_dram_parameter("iota", [P, P], BF16, isOutput=False)
    houtT = nc.declare_dram_parameter("houtT", [CLSP, CH * P], F32, isOutput=True)

    # resident-g2 slab boundaries: ~4 big DMAs aligned to chunk starts
    slab_bounds = [0]
    tgt_full = (T + 7) // 8
    for k in range(CH):
        t_end = int(mt.tile_off[k]) + int(mt.tiles_k[k])
        n_so_far = len(slab_bounds) - 1
        tgt = tgt_full // 4 if n_so_far < 2 else (tgt_full // 2 if n_so_far < 3 else tgt_full)
        if (t_end - slab_bounds[-1] >= tgt or k == CH - 1) and t_end > slab_bounds[-1]:
            slab_bounds.append(t_end)
    chunk_slab = {}
    for k in range(CH):
        t0 = int(mt.tile_off[k])
        chunk_slab[k] = sum(1 for b in slab_bounds[1:-1] if b <= t0)

    with TileContext(nc) as tc:
        with (
            tc.tile_pool(name="res", bufs=1) as res,
            tc.tile_pool(name="gwp", bufs=4) as gwp,
            tc.tile_pool(name="ohp", bufs=4) as ohp,
            tc.tile_pool(name="epi", bufs=3) as epi,
            tc.tile_pool(name="pso", bufs=3, space="PSUM") as pso,
        ):
            iota_sb = res.tile([P, P], BF16)
            nc.scalar.dma_start(out=iota_sb[:], in_=iota_in[:])
            dexp_sb = res.tile([P, T, 2], BF16)
            nc.scalar.dma_start(
                out=dexp_sb[:],
                in_=dexp_in[:].rearrange("p (t two) -> p t two", two=2),
            )
            aexp_sb = res.tile([P, T, 2], BF16)
            nc.scalar.dma_start(
                out=aexp_sb[:],
                in_=aexp_in[:].rearrange("p (t two) -> p t two", two=2),
            )
            g2_slabs = []
            for si in range(len(slab_bounds) - 1):
                a, b = slab_bounds[si], slab_bounds[si + 1]
                sl = res.tile([P, b - a, CLSP], BF16, name=f"g2sl{si}", tag=f"g2sl{si}")
                nc.sync.dma_start(
                    out=sl[:],
                    in_=g2_in[:, a * CLSP : b * CLSP].rearrange(
                        "p (t e) -> p t e", e=CLSP
                    ),
                )
                g2_slabs.append(sl)

            for k in range(CH):
                nt = int(mt.tiles_k[k])
                t0 = int(mt.tile_off[k])
                si = chunk_slab[k]
                g2_sb = g2_slabs[si][:, t0 - slab_bounds[si] : t0 - slab_bounds[si] + nt, :]
                # g2w = g2 * alpha2 (one DVE op per chunk, 2x mode)
                g2w = gwp.tile([P, nt, CLSP], BF16, tag="g2w")
                a_ap = aexp_sb[:, t0 : t0 + nt, :]
                a_bc = bass.AP(
                    a_ap.tensor,
                    a_ap.offset,
                    [list(a_ap.ap[0]), [2, nt], [0, CLSP // 2], [1, 2]],
                )
                gview = lambda tile: bass.AP(
                    tile.tensor,
                    tile.offset,
                    [list(tile.ap[0]), [CLSP, nt], [2, CLSP // 2], [1, 2]],
                )
                nc.vector.tensor_tensor(
                    out=gview(g2w[:, :, :]),
                    in0=gview(g2_sb[:, :, :]),
                    in1=a_bc,
                    op=mybir.AluOpType.mult,
                )
                oh = ohp.tile([P, nt, P], BF16, tag="oh")
                _onehot_batch(nc, oh, iota_sb, dexp_sb, t0, nt)
                o_ps = pso.tile([CLSP, P], F32, tag="o")
                for j in range(nt):
                    nc.tensor.matmul(
                        o_ps[:],
                        lhsT=g2w[:, j, :],
                        rhs=oh[:, j, :],
                        start=(j == 0),
                        stop=(j == nt - 1),
                        skip_group_check=True,
                    )
                o_sb = epi.tile([CLSP, P], F32, tag="osb")
                nc.scalar.activation(
                    o_sb[:], o_ps[:], mybir.ActivationFunctionType.Copy
                )
                nc.sync.dma_start(out=houtT[:, k * P : (k + 1) * P], in_=o_sb[:])
    nc.compile()
    _split_sync_waits(nc)
    return nc


def kernel(
    x,
    edge_index,
    W1,
    att_src1,
    att_dst1,
    b1,
    W2,
    att_src2,
    att_dst2,
    b2,
    _trace=False,
    _tmpdirs=None,
):
    x = np.asarray(x, dtype=np.float32)
    edge_index = np.asarray(edge_index).astype(np.int64)
    W1 = np.asarray(W1, dtype=np.float32)
    att_src1 = np.asarray(att_src1, dtype=np.float32)
    att_dst1 = np.asarray(att_dst1, dtype=np.float32)
    b1 = np.asarray(b1, dtype=np.float32)
    W2 = np.asarray(W2, dtype=np.float32)
    att_src2 = np.asarray(att_src2, dtype=np.float32)
    att_dst2 = np.asarray(att_dst2, dtype=np.float32)
    b2 = np.asarray(b2, dtype=np.float32)

    N, F_IN = x.shape
    HEADS, HID = att_src1.shape
    CLS = W2.shape[1]
    W2E = ((CLS + 2 + 15) // 16) * 16
    CLSP = ((CLS + 15) // 16) * 16

    key = (N, edge_index.shape[1], F_IN, HEADS, HID, CLS, hash(edge_index.tobytes()))
    if key in _CACHE:
        mt, ncA, ncB = _CACHE[key]
    else:
        mt = _preprocess(N, edge_index)
        ncA = _build_p1(mt, F_IN, HEADS, HID, CLS)
        ncB = _build_p2(mt, CLS)
        _CACHE[key] = (mt, ncA, ncB)

    NLOC, CH, T, S = mt.NLOC, mt.CH, mt.T, mt.S

    # ---- host: layer-1 alpha (a_s/a_d are linear in x) ----
    W1r = W1.reshape(F_IN, HEADS, HID)
    v_s = np.einsum("fhc,hc->fh", W1r, att_src1)
    v_d = np.einsum("fhc,hc->fh", W1r, att_dst1)
    a_s = x.astype(np.float64) @ v_s.astype(np.float64)
    a_d = x.astype(np.float64) @ v_d.astype(np.float64)
    z1 = _leaky(a_s[mt.src_s] + a_d[mt.dst_s])
    alpha1 = _seg_softmax(z1, mt.dst_s, N)

    aexp = _alpha_plane_pairs(mt, alpha1)  # [C,128,T,H,2] bf16
    iota = np.tile(np.arange(P, dtype=np.float32)[None, :], (P, 1)).astype(BF)
    dexp = np.ascontiguousarray(
        np.repeat(mt.dpos_plane[..., None], 2, axis=-1)
    ).astype(BF)  # [C,128,T,2]

    # xgT tiles: [C, 128 feat, S] bf16 = x.T columns at slot srcs
    xT16 = np.ascontiguousarray(x.astype(BF).T)  # [F_IN, N]
    w1b = W1.astype(BF)
    v_s2 = (W2 @ att_src2[0]).astype(np.float32)
    v_d2 = (W2 @ att_dst2[0]).astype(np.float32)
    w2e = np.zeros((HEADS * HID, W2E), np.float32)
    w2e[:, 0] = v_s2
    w2e[:, 1] = v_d2
    w2e[:, 2 : 2 + CLS] = W2
    w2eb = w2e.astype(BF)
    b1col = b1.reshape(P, 1).astype(np.float32)

    in_maps_a = []
    for c in range(N_CORES):
        xg = np.ascontiguousarray(xT16[:, mt.slot_src[c]])  # [128, S]
        in_maps_a.append(
            {
                "xg": xg,
                "aexp": np.ascontiguousarray(aexp[c].reshape(P, -1)),
                "dexp": np.ascontiguousarray(dexp[c].reshape(P, -1)),
                "iota": iota,
                "w1": w1b,
                "w2e": w2eb,
                "b1col": b1col,
            }
        )

    tds = _tmpdirs or [None, None]
    resA = run_bass_kernel_spmd(
        ncA, in_maps_a, list(range(N_CORES)), trace=_trace, tmpdir=tds[0]
    )

    # host: assemble tab2 + a_s2/a_d2, compute alpha2
    tab2 = np.zeros((N, CLSP), BF)
    asd = np.zeros((N, 2), np.float64)
    for c in range(N_CORES):
        hT = np.asarray(resA.results[c]["houtT"])  # [W2E, CH*P] bf16
        tab2[c * NLOC : (c + 1) * NLOC, :CLS] = hT[2 : 2 + CLS, :NLOC].T
        asd[c * NLOC : (c + 1) * NLOC] = np.asarray(
            resA.results[c]["asdT"], np.float64
        )[:, :NLOC].T

    z2 = _leaky(asd[mt.src_s, 0] + asd[mt.dst_s, 1])[:, None]
    alpha2 = _seg_softmax(z2, mt.dst_s, N)
    a2exp = _alpha_plane_pairs(mt, alpha2)  # [C,128,T,1,2]

    in_maps_b = []
    for c in range(N_CORES):
        g2 = tab2[mt.slot_src[c]]  # [S, CLSP] bf16
        g2 = np.ascontiguousarray(
            g2.reshape(T, P, CLSP).transpose(1, 0, 2).reshape(P, T * CLSP)
        )
        in_maps_b.append(
            {
                "g2": g2,
                "aexp": np.ascontiguousarray(a2exp[c].reshape(P, -1)),
                "dexp": np.ascontiguousarray(dexp[c].reshape(P, -1)),
                "iota": iota,
            }
        )

    resB = run_bass_kernel_spmd(
        ncB, in_maps_b, list(range(N_CORES)), trace=_trace, tmpdir=tds[1]
    )

    out = np.zeros((N, CLS), np.float32)
    for c in range(N_CORES):
        out[c * NLOC : (c + 1) * NLOC] = np.asarray(
            resB.results[c]["houtT"], np.float32
        )[:CLS, :NLOC].T
    out += b2[None, :]

    kernel._last = (resA, resB)
    return out


# revision 37
# speedup vs baseline: 1.3043x; 1.0040x over previous
"""GAT (2-layer, PyG-style) on 8 Trainium2 NeuronCores.

Strategy (v2 — host-staged gather, two collective-free device programs)
----------------------------------------------------------------------
- Nodes are sharded across the 8 cores by dst (N/8 rows each). Edges are
  sorted by dst and packed into 128-dst windows; each window's edges are
  padded to 128-edge tiles (slot layout identical on all cores; per-core
  counts only differ in the padding).
- The host stages the per-edge operand tiles (the "gather"):
    program P1 gets xgT tiles  — x[src_e] columns, [128 feat x 128 edge],
    program P2 gets g2 tiles   — tab2[src_e] rows, [128 edge x CLS],
  and the per-slot alpha/dst-position planes. Attention coefficients are
  computed on the host exactly as in the v1 kernel (layer-1 alpha is a
  pure function of the inputs; layer-2 alpha comes from per-node a_s2 /
  a_d2 scalars returned by P1).
- Program P1, per 128-edge tile:
    g   = xgT_tile^T @ W1            (PE, psum)
    g4  = copy psum->sbuf bf16       (ACT, batched over 4 tiles)
    gw  = g4 * alpha1                (DVE tensor_tensor, pair-expanded
                                      alpha plane to hit the 2x mode)
    oh  = (iota == dpos)             (DVE tensor_scalar, 4x mode)
    aggT += gw^T @ oh                (PE, psum accum per 128-dst window)
  and per window: h1T = Relu(aggT + b1) (ACT, bias per partition), then
  tab2T = [v_s2 | v_d2 | W2]^T @ h1T (PE) which is written out: rows
  0, 1 are a_s2 / a_d2, rows 2..CLS+1 are the layer-2 table.
- Program P2, per tile: alpha-one-hot via one fused tensor_scalar
  ((iota == dpos) * alpha2), then out[d,:] += ahot^T @ g2_tile (PE).
  Host adds b2 and reassembles the full output.

All engine work stays on device; the host does input marshalling
(edge-index bookkeeping, fancy-indexed tile staging) and the softmax
normalization of the attention logits, as in v1.

Self-contained: all shapes/structure are derived from the actual inputs.
"""

import numpy as np
import ml_dtypes

import bass_rust
import concourse.bass as bass
import concourse.bacc as bacc
import concourse.mybir as mybir
from concourse.bass_utils import run_bass_kernel_spmd
from concourse.tile import TileContext, ScopedClock

# ----------------------------------------------------------------------------
# Workaround: this walrus build rejects >1 sync wait on a CTRL op, but the
# stock TileContext tail drain carries one wait per live proc. Split them
# across nofuse NOPs (one wait each).
# ----------------------------------------------------------------------------


def _patched_drain_and_barrier(self, tick_clock, wait_clock):
    nc = self.nc
    probe = nc.sync.nop(nofuse=True, hint="tail_drain_waits")
    wait_clock.add_sem_waits(probe.ins, ScopedClock({None: tick_clock.global_clock}))
    si = probe.ins.sync_info
    waits = list(si.on_wait) if si is not None else []
    if len(waits) > 1:
        probe.ins.sync_info = bass_rust.SyncInfo(on_wait=waits[:1], on_update=[])
        for i in range(1, len(waits)):
            n = nc.sync.nop(nofuse=True, hint=f"tail_drain_waits_{i}")
            n.ins.sync_info = bass_rust.SyncInfo(on_wait=waits[i : i + 1], on_update=[])
    nc.sync.drain()
    nc.all_engine_barrier()
    assert self.sems is not None
    popped = nc._tile_sem_poison_stack.pop()
    assert popped is self._sem_poison
    nc.clear_and_free_semaphores(list(self.sems.allocated().values()))
    nc.all_engine_barrier()


TileContext._drain_and_barrier = _patched_drain_and_barrier

MAX_WAITS = 1  # this walrus build rejects instructions with more sync waits


def _split_sync_waits(nc, max_waits=MAX_WAITS):
    """Hoist excess per-instruction sync waits onto standalone nofuse NOPs
    placed immediately before the instruction (same engine)."""
    n_new = 0
    for bbname, bassbb in list(nc._state.bb_map.items()):
        bb = bassbb.bb
        insts = list(bb.instructions)
        out = []
        changed = False
        for inst in insts:
            si = inst.sync_info
            if si is not None and len(si.on_wait) > max_waits:
                waits = list(si.on_wait)
                extra = waits[:-max_waits]
                for j in range(0, len(extra), max_waits):
                    nop = mybir.InstNoOp(
                        name=f"{inst.name}-w{n_new}",
                        engine=inst.engine,
                        bass_nofuse=True,
                        sync_info=bass_rust.SyncInfo(
                            on_wait=extra[j : j + max_waits], on_update=[]
                        ),
                    )
                    n_new += 1
                    nc.register_instruction(nop, overwrite=True)
                    out.append(nop)
                inst.sync_info = bass_rust.SyncInfo(
                    on_wait=waits[-max_waits:], on_update=list(si.on_update)
                )
                changed = True
            out.append(inst)
        if changed:
            bb.instructions = out
    return n_new

# ----------------------------------------------------------------------------

P = 128
N_CORES = 8
NEG_SLOPE = 0.2
BATCH = 8  # tiles per psum batch in P1 (psum tile spans 2 banks)

F32 = mybir.dt.float32
BF16 = mybir.dt.bfloat16
BF = ml_dtypes.bfloat16

_CACHE = {}


def _leaky(z):
    return np.where(z > 0, z, NEG_SLOPE * z)


def _seg_softmax(z, dst, n):
    """Exact segment softmax over sorted dst (every dst has >=1 edge)."""
    starts = np.searchsorted(dst, np.arange(n))
    m = np.maximum.reduceat(z, starts, axis=0)
    w = np.exp(z - m[dst])
    den = np.add.reduceat(w, starts, axis=0)
    return w / den[dst]


class _Meta:
    pass


def _preprocess(N, edge_index):
    """Sort edges by dst, shard by dst range, build the static window/tile
    slot structure shared by both device programs (identical on all cores;
    per-core data differs only in padding)."""
    mt = _Meta()
    assert N % N_CORES == 0
    NLOC = N // N_CORES
    CH = (NLOC + P - 1) // P
    mt.N, mt.NLOC, mt.CH = N, NLOC, CH
    mt.SH_PAD = CH * P

    src = np.concatenate([edge_index[0], np.arange(N, dtype=np.int64)])
    dst = np.concatenate([edge_index[1], np.arange(N, dtype=np.int64)])
    order = np.argsort(dst, kind="stable")
    mt.src_s, mt.dst_s = src[order], dst[order]
    E = src.shape[0]
    mt.E = E

    # per-(core, window) edge ranges
    cnt = np.zeros((N_CORES, CH), dtype=np.int64)
    rng = np.zeros((N_CORES, CH, 2), dtype=np.int64)
    for c in range(N_CORES):
        for k in range(CH):
            d0 = c * NLOC + k * P
            d1 = min(c * NLOC + min((k + 1) * P, NLOC), N)
            s = np.searchsorted(mt.dst_s, d0)
            e = np.searchsorted(mt.dst_s, d1)
            rng[c, k] = (s, e)
            cnt[c, k] = e - s

    tiles_k = ((cnt.max(axis=0) + P - 1) // P).astype(np.int64)  # per window
    mt.tiles_k = tiles_k
    mt.T = int(tiles_k.sum())
    mt.S = mt.T * P
    mt.tile_off = np.concatenate([[0], np.cumsum(tiles_k)])[:-1]  # tile idx of win k

    # per-core slot arrays
    slot_src = np.zeros((N_CORES, mt.S), dtype=np.int64)
    slot_eid = np.full((N_CORES, mt.S), -1, dtype=np.int64)
    slot_dpos = np.full((N_CORES, mt.S), -1.0, dtype=np.float64)
    for c in range(N_CORES):
        for k in range(CH):
            s, e = rng[c, k]
            ne = e - s
            s0 = int(mt.tile_off[k]) * P
            slot_src[c, s0 : s0 + ne] = mt.src_s[s:e]
            slot_eid[c, s0 : s0 + ne] = np.arange(s, e)
            slot_dpos[c, s0 : s0 + ne] = mt.dst_s[s:e] - (c * NLOC + k * P)
    mt.slot_src = slot_src
    mt.slot_eid = slot_eid

    # dpos plane [C, 128, T] f32: slot j -> [j%128, j//128]
    mt.dpos_plane = np.ascontiguousarray(
        slot_dpos.reshape(N_CORES, mt.T, P).transpose(0, 2, 1)
    ).astype(np.float32)
    return mt


def _alpha_plane_pairs(mt, alpha):
    """alpha [E, H] (dst-sorted edge order) -> [C, 128, T, H, 2] bf16 slot
    planes with each value duplicated in pairs (so the DVE 2x mode applies)."""
    H = alpha.shape[1]
    eid = mt.slot_eid
    valid = eid >= 0
    vals = np.zeros((N_CORES, mt.S, H), dtype=np.float32)
    vals[valid] = alpha[eid[valid]].astype(np.float32)
    out = vals.reshape(N_CORES, mt.T, P, H).transpose(0, 2, 1, 3)  # [C,128,T,H]
    out = np.repeat(out[..., None], 2, axis=-1)  # [C,128,T,H,2]
    return np.ascontiguousarray(out).astype(BF)


def _onehot_batch(nc, oh_sb, iota_sb, dexp_sb, t0, nt):
    """oh[:, j, d] = (iota[d] == dpos[:, t0+j]) for j in [0, nt), one DVE op.
    All operands viewed with a trailing [1,2] pair dim so the 2x mode kicks
    in (dexp is the pair-duplicated dpos plane)."""
    d_ap = dexp_sb[:, t0 : t0 + nt, :]
    d_bc = bass.AP(
        d_ap.tensor,
        d_ap.offset,
        [list(d_ap.ap[0]), [2, nt], [0, P // 2], [1, 2]],
    )
    oh_ap = oh_sb[:, :nt, :]
    oh_v = bass.AP(
        oh_ap.tensor,
        oh_ap.offset,
        [list(oh_ap.ap[0]), [P, nt], [2, P // 2], [1, 2]],
    )
    io_ap = iota_sb[:]
    io_bc = bass.AP(
        io_ap.tensor,
        io_ap.offset,
        [list(io_ap.ap[0]), [0, nt], [2, P // 2], [1, 2]],
    )
    nc.vector.tensor_tensor(
        out=oh_v, in0=io_bc, in1=d_bc, op=mybir.AluOpType.is_equal
    )


def _build_p1(mt, F_IN, HEADS, HID, CLS):
    """Layer-1 program: xgT tiles -> h1 windows -> tab2T/asd output."""
    F_HID = HEADS * HID
    CH, T, S = mt.CH, mt.T, mt.S
    W2E = ((CLS + 2 + 15) // 16) * 16  # padded [v_s2 | v_d2 | W2] columns
    assert F_IN == P and F_HID == P

    nc = bacc.Bacc("TRN2", target_bir_lowering=False, debug=False, num_devices=N_CORES)
    xg_in = nc.declare_dram_parameter("xg", [P, S], BF16, isOutput=False)
    aexp_in = nc.declare_dram_parameter("aexp", [P, T * HEADS * 2], BF16, isOutput=False)
    dexp_in = nc.declare_dram_parameter("dexp", [P, T * 2], BF16, isOutput=False)
    iota_in = nc.declare_dram_parameter("iota", [P, P], BF16, isOutput=False)
    w1_in = nc.declare_dram_parameter("w1", [P, F_HID], BF16, isOutput=False)
    w2e_in = nc.declare_dram_parameter("w2e", [P, W2E], BF16, isOutput=False)
    b1_in = nc.declare_dram_parameter("b1col", [P, 1], F32, isOutput=False)
    houtT = nc.declare_dram_parameter("houtT", [W2E, CH * P], BF16, isOutput=True)
    asdT = nc.declare_dram_parameter("asdT", [2, CH * P], F32, isOutput=True)

    # streamed-xg slab boundaries: ~17 DMAs, 4 rotating SBUF slots; the first
    # slabs are small so the first chunk's data lands quickly.
    slab_bounds = [0]
    tgt_full = (T + 15) // 16
    for k in range(CH):
        t_end = int(mt.tile_off[k]) + int(mt.tiles_k[k])
        n_so_far = len(slab_bounds) - 1
        tgt = tgt_full // 4 if n_so_far < 2 else (tgt_full // 2 if n_so_far < 3 else tgt_full)
        if (t_end - slab_bounds[-1] >= tgt or k == CH - 1) and t_end > slab_bounds[-1]:
            slab_bounds.append(t_end)
    chunk_slab = {}
    for k in range(CH):
        t0 = int(mt.tile_off[k])
        chunk_slab[k] = sum(1 for b in slab_bounds[1:-1] if b <= t0)

    with TileContext(nc) as tc:
        with (
            tc.tile_pool(name="res", bufs=1) as res,
            tc.tile_pool(name="g4p", bufs=4) as g4p,
            tc.tile_pool(name="gwp", bufs=4) as gwp,
            tc.tile_pool(name="ohp", bufs=4) as ohp,
            tc.tile_pool(name="epi", bufs=3) as epi,
            tc.tile_pool(name="psg", bufs=2, space="PSUM") as psg,
            tc.tile_pool(name="psa", bufs=2, space="PSUM") as psa,
            tc.tile_pool(name="pst", bufs=2, space="PSUM") as pst,
        ):
            # residents on the ACT HWDGE ring; data slabs on the SP ring, so
            # the first slab isn't queued behind the big alpha plane.
            w1_sb = res.tile([P, F_HID], BF16)
            nc.scalar.dma_start(out=w1_sb[:], in_=w1_in[:])
            iota_sb = res.tile([P, P], BF16)
            nc.scalar.dma_start(out=iota_sb[:], in_=iota_in[:])
            b1_sb = res.tile([P, 1], F32)
            nc.scalar.dma_start(out=b1_sb[:], in_=b1_in[:])
            dexp_sb = res.tile([P, T, 2], BF16)
            nc.scalar.dma_start(
                out=dexp_sb[:],
                in_=dexp_in[:].rearrange("p (t two) -> p t two", two=2),
            )
            aexp_sb = res.tile([P, T, HEADS, 2], BF16)
            nc.scalar.dma_start(
                out=aexp_sb[:],
                in_=aexp_in[:].rearrange("p (t h two) -> p t h two", h=HEADS, two=2),
            )
            w2e_sb = res.tile([P, W2E], BF16)
            nc.scalar.dma_start(out=w2e_sb[:], in_=w2e_in[:])

            max_slab = max(
                slab_bounds[i + 1] - slab_bounds[i]
                for i in range(len(slab_bounds) - 1)
            )
            xg_slabs = []
            for si in range(len(slab_bounds) - 1):
                a, b = slab_bounds[si], slab_bounds[si + 1]
                sl = res.tile(
                    [P, max_slab, P], BF16, name=f"xgsl{si}", tag=f"xgsl{si % 4}"
                )[:, : b - a, :]
                nc.sync.dma_start(
                    out=sl[:],
                    in_=xg_in[:, a * P : b * P].rearrange("p (t e) -> p t e", e=P),
                )
                xg_slabs.append(sl)

            for k in range(CH):
                nt = int(mt.tiles_k[k])
                t0 = int(mt.tile_off[k])
                si = chunk_slab[k]
                xg_sb = xg_slabs[si][
                    :, t0 - slab_bounds[si] : t0 - slab_bounds[si] + nt, :
                ]
                agg_ps = psa.tile([P, P], F32, tag="agg")
                g_sb = g4p.tile([P, nt, P], BF16, tag="g")
                n_batches = (nt + BATCH - 1) // BATCH
                for b in range(n_batches):
                    j0 = b * BATCH
                    nb = min(BATCH, nt - j0)
                    g_ps = psg.tile([P, BATCH * P], F32, tag="gps")
                    for j in range(nb):
                        nc.tensor.matmul(
                            g_ps[:, j * P : (j + 1) * P],
                            lhsT=xg_sb[:, j0 + j, :],
                            rhs=w1_sb[:],
                            start=True,
                            stop=True,
                            skip_group_check=True,
                        )
                    nc.scalar.activation(
                        g_sb[:, j0 : j0 + nb, :],
                        g_ps[:, : nb * P].rearrange("p (t e) -> p t e", e=P),
                        mybir.ActivationFunctionType.Copy,
                    )
                # gw = g * alpha, split at the psum-batch boundary so the
                # aggregation matmuls start before the last copy lands
                gw = gwp.tile([P, nt, P], BF16, tag="gw")
                def gview(tile, lo, n):
                    sl = tile[:, lo : lo + n, :]
                    return bass.AP(
                        sl.tensor,
                        sl.offset,
                        [list(sl.ap[0]), [HID, n * HEADS], [2, HID // 2], [1, 2]],
                    )
                mult_splits = [(0, min(BATCH, nt))]
                if nt > BATCH:
                    mult_splits.append((BATCH, nt - BATCH))
                for lo, n in mult_splits:
                    a_ap = aexp_sb[:, t0 + lo : t0 + lo + n, :, :]
                    a_bc = bass.AP(
                        a_ap.tensor,
                        a_ap.offset,
                        [list(a_ap.ap[0]), [2, n * HEADS], [0, HID // 2], [1, 2]],
                    )
                    nc.vector.tensor_tensor(
                        out=gview(gw, lo, n),
                        in0=gview(g_sb, lo, n),
                        in1=a_bc,
                        op=mybir.AluOpType.mult,
                    )
                oh = ohp.tile([P, nt, P], BF16, tag="oh")
                _onehot_batch(nc, oh, iota_sb, dexp_sb, t0, nt)
                for j in range(nt):
                    nc.tensor.matmul(
                        agg_ps[:],
                        lhsT=gw[:, j, :],
                        rhs=oh[:, j, :],
                        start=(j == 0),
                        stop=(j == nt - 1),
                        skip_group_check=True,
                    )
                # epilogue: h1T = relu(aggT + b1), tab2T = W2e^T @ h1T
                h1T = epi.tile([P, P], BF16, tag="h1T")
                nc.scalar.activation(
                    h1T[:],
                    agg_ps[:],
                    mybir.ActivationFunctionType.Relu,
                    bias=b1_sb[:, 0:1],
                )
                t2_ps = pst.tile([W2E, P], F32, tag="t2")
                nc.tensor.matmul(
                    t2_ps[:],
                    lhsT=w2e_sb[:],
                    rhs=h1T[:],
                    start=True,
                    stop=True,
                    skip_group_check=True,
                )
                t2_sb = epi.tile([W2E, P], BF16, tag="t2sb")
                nc.vector.tensor_copy(out=t2_sb[:], in_=t2_ps[:])
                asd_sb = epi.tile([2, P], F32, tag="asd")
                nc.vector.tensor_copy(out=asd_sb[:], in_=t2_ps[0:2, :])
                nc.sync.dma_start(out=houtT[:, k * P : (k + 1) * P], in_=t2_sb[:])
                nc.sync.dma_start(out=asdT[:, k * P : (k + 1) * P], in_=asd_sb[:])
    nc.compile()
    _split_sync_waits(nc)
    return nc


def _build_p2(mt, CLS):
    """Layer-2 program: g2 tiles * alpha2 aggregated per window (transposed
    matmul so the PE streams 128 columns and leaves the cold p-state)."""
    CH, T, S = mt.CH, mt.T, mt.S
    CLSP = ((CLS + 15) // 16) * 16

    nc = bacc.Bacc("TRN2", target_bir_lowering=False, debug=False, num_devices=N_CORES)
    g2_in = nc.declare_dram_parameter("g2", [P, T * CLSP], BF16, isOutput=False)
    aexp_in = nc.declare_dram_parameter("aexp", [P, T * 2], BF16, isOutput=False)
    dexp_in = nc.declare_dram_parameter("dexp", [P, T * 2], BF16, isOutput=False)
    iota_in = nc.declare_dram_parameter("iota", [P, P], BF16, isOutput=False)
    houtT = nc.declare_dram_parameter("houtT", [CLSP, CH * P], F32, isOutput=True)

    # resident-g2 slab boundaries: ~4 big DMAs aligned to chunk starts
    slab_bounds = [0]
    tgt_full = (T + 7) // 8
    for k in range(CH):
        t_end = int(mt.tile_off[k]) + int(mt.tiles_k[k])
        n_so_far = len(slab_bounds) - 1
        tgt = tgt_full // 4 if n_so_far < 2 else (tgt_full // 2 if n_so_far < 3 else tgt_full)
        if (t_end - slab_bounds[-1] >= tgt or k == CH - 1) and t_end > slab_bounds[-1]:
            slab_bounds.append(t_end)
    chunk_slab = {}
    for k in range(CH):
        t0 = int(mt.tile_off[k])
        chunk_slab[k] = sum(1 for b in slab_bounds[1:-1] if b <= t0)

    with TileContext(nc) as tc:
        with (
            tc.tile_pool(name="res", bufs=1) as res,
            tc.tile_pool(name="gwp", bufs=4) as gwp,
            tc.tile_pool(name="ohp", bufs=4) as ohp,
            tc.tile_pool(name="epi", bufs=3) as epi,
            tc.tile_pool(name="pso", bufs=3, space="PSUM") as pso,
        ):
            iota_sb = res.tile([P, P], BF16)
            nc.scalar.dma_start(out=iota_sb[:], in_=iota_in[:])
            dexp_sb = res.tile([P, T, 2], BF16)
            nc.scalar.dma_start(
                out=dexp_sb[:],
                in_=dexp_in[:].rearrange("p (t two) -> p t two", two=2),
            )
            aexp_sb = res.tile([P, T, 2], BF16)
            nc.scalar.dma_start(
                out=aexp_sb[:],
                in_=aexp_in[:].rearrange("p (t two) -> p t two", two=2),
            )
            g2_slabs = []
            for si in range(len(slab_bounds) - 1):
                a, b = slab_bounds[si], slab_bounds[si + 1]
                sl = res.tile([P, b - a, CLSP], BF16, name=f"g2sl{si}", tag=f"g2sl{si}")
                nc.sync.dma_start(
                    out=sl[:],
                    in_=g2_in[:, a * CLSP : b * CLSP].rearrange(
                        "p (t e) -> p t e", e=CLSP
                    ),
                )
                g2_slabs.append(sl)

            for k in range(CH):
                nt = int(mt.tiles_k[k])
                t0 = int(mt.tile_off[k])
                si = chunk_slab[k]
                g2_sb = g2_slabs[si][:, t0 - slab_bounds[si] : t0 - slab_bounds[si] + nt, :]
                # g2w = g2 * alpha2 (one DVE op per chunk, 2x mode)
                g2w = gwp.tile([P, nt, CLSP], BF16, tag="g2w")
                a_ap = aexp_sb[:, t0 : t0 + nt, :]
                a_bc = bass.AP(
                    a_ap.tensor,
                    a_ap.offset,
                    [list(a_ap.ap[0]), [2, nt], [0, CLSP // 2], [1, 2]],
                )
                gview = lambda tile: bass.AP(
                    tile.tensor,
                    tile.offset,
                    [list(tile.ap[0]), [CLSP, nt], [2, CLSP // 2], [1, 2]],
                )
                nc.vector.tensor_tensor(
                    out=gview(g2w[:, :, :]),
                    in0=gview(g2_sb[:, :, :]),
                    in1=a_bc,
                    op=mybir.AluOpType.mult,
                )
                oh = ohp.tile([P, nt, P], BF16, tag="oh")
                _onehot_batch(nc, oh, iota_sb, dexp_sb, t0, nt)
                o_ps = pso.tile([CLSP, P], F32, tag="o")
                for j in range(nt):
                    nc.tensor.matmul(
                        o_ps[:],
                        lhsT=g2w[:, j, :],
                        rhs=oh[:, j, :],
                        start=(j == 0),
                        stop=(j == nt - 1),
                        skip_group_check=True,
                    )
                o_sb = epi.tile([CLSP, P], F32, tag="osb")
                nc.scalar.activation(
                    o_sb[:], o_ps[:], mybir.ActivationFunctionType.Copy
                )
                nc.sync.dma_start(out=houtT[:, k * P : (k + 1) * P], in_=o_sb[:])
    nc.compile()
    _split_sync_waits(nc)
    return nc


def kernel(
    x,
    edge_index,
    W1,
    att_src1,
    att_dst1,
    b1,
    W2,
    att_src2,
    att_dst2,
    b2,
    _trace=False,
    _tmpdirs=None,
):
    x = np.asarray(x, dtype=np.float32)
    edge_index = np.asarray(edge_index).astype(np.int64)
    W1 = np.asarray(W1, dtype=np.float32)
    att_src1 = np.asarray(att_src1, dtype=np.float32)
    att_dst1 = np.asarray(att_dst1, dtype=np.float32)
    b1 = np.asarray(b1, dtype=np.float32)
    W2 = np.asarray(W2, dtype=np.float32)
    att_src2 = np.asarray(att_src2, dtype=np.float32)
    att_dst2 = np.asarray(att_dst2, dtype=np.float32)
    b2 = np.asarray(b2, dtype=np.float32)

    N, F_IN = x.shape
    HEADS, HID = att_src1.shape
    CLS = W2.shape[1]
    W2E = ((CLS + 2 + 15) // 16) * 16
    CLSP = ((CLS + 15) // 16) * 16

    key = (N, edge_index.shape[1], F_IN, HEADS, HID, CLS, hash(edge_index.tobytes()))
    if key in _CACHE:
        mt, ncA, ncB = _CACHE[key]
    else:
        mt = _preprocess(N, edge_index)
        ncA = _build_p1(mt, F_IN, HEADS, HID, CLS)
        ncB = _build_p2(mt, CLS)
        _CACHE[key] = (mt, ncA, ncB)

    NLOC, CH, T, S = mt.NLOC, mt.CH, mt.T, mt.S

    # ---- host: layer-1 alpha (a_s/a_d are linear in x) ----
    W1r = W1.reshape(F_IN, HEADS, HID)
    v_s = np.einsum("fhc,hc->fh", W1r, att_src1)
    v_d = np.einsum("fhc,hc->fh", W1r, att_dst1)
    a_s = x.astype(np.float64) @ v_s.astype(np.float64)
    a_d = x.astype(np.float64) @ v_d.astype(np.float64)
    z1 = _leaky(a_s[mt.src_s] + a_d[mt.dst_s])
    alpha1 = _seg_softmax(z1, mt.dst_s, N)

    aexp = _alpha_plane_pairs(mt, alpha1)  # [C,128,T,H,2] bf16
    iota = np.tile(np.arange(P, dtype=np.float32)[None, :], (P, 1)).astype(BF)
    dexp = np.ascontiguousarray(
        np.repeat(mt.dpos_plane[..., None], 2, axis=-1)
    ).astype(BF)  # [C,128,T,2]

    # xgT tiles: [C, 128 feat, S] bf16 = x.T columns at slot srcs
    xT16 = np.ascontiguousarray(x.astype(BF).T)  # [F_IN, N]
    w1b = W1.astype(BF)
    v_s2 = (W2 @ att_src2[0]).astype(np.float32)
    v_d2 = (W2 @ att_dst2[0]).astype(np.float32)
    w2e = np.zeros((HEADS * HID, W2E), np.float32)
    w2e[:, 0] = v_s2
    w2e[:, 1] = v_d2
    w2e[:, 2 : 2 + CLS] = W2
    w2eb = w2e.astype(BF)
    b1col = b1.reshape(P, 1).astype(np.float32)

    in_maps_a = []
    for c in range(N_CORES):
        xg = np.ascontiguousarray(xT16[:, mt.slot_src[c]])  # [128, S]
        in_maps_a.append(
            {
                "xg": xg,
                "aexp": np.ascontiguousarray(aexp[c].reshape(P, -1)),
                "dexp": np.ascontiguousarray(dexp[c].reshape(P, -1)),
                "iota": iota,
                "w1": w1b,
                "w2e": w2eb,
                "b1col": b1col,
            }
        )

    tds = _tmpdirs or [None, None]
    resA = run_bass_kernel_spmd(
        ncA, in_maps_a, list(range(N_CORES)), trace=_trace, tmpdir=tds[0]
    )

    # host: assemble tab2 + a_s2/a_d2, compute alpha2
    tab2 = np.zeros((N, CLSP), BF)
    asd = np.zeros((N, 2), np.float64)
    for c in range(N_CORES):
        hT = np.asarray(resA.results[c]["houtT"])  # [W2E, CH*P] bf16
        tab2[c * NLOC : (c + 1) * NLOC, :CLS] = hT[2 : 2 + CLS, :NLOC].T
        asd[c * NLOC : (c + 1) * NLOC] = np.asarray(
            resA.results[c]["asdT"], np.float64
        )[:, :NLOC].T

    z2 = _leaky(asd[mt.src_s, 0] + asd[mt.dst_s, 1])[:, None]
    alpha2 = _seg_softmax(z2, mt.dst_s, N)
    a2exp = _alpha_plane_pairs(mt, alpha2)  # [C,128,T,1,2]

    in_maps_b = []
    for c in range(N_CORES):
        g2 = tab2[mt.slot_src[c]]  # [S, CLSP] bf16
        g2 = np.ascontiguousarray(
            g2.reshape(T, P, CLSP).transpose(1, 0, 2).reshape(P, T * CLSP)
        )
        in_maps_b.append(
            {
                "g2": g2,
                "aexp": np.ascontiguousarray(a2exp[c].reshape(P, -1)),
                "dexp": np.ascontiguousarray(dexp[c].reshape(P, -1)),
                "iota": iota,
            }
        )

    resB = run_bass_kernel_spmd(
        ncB, in_maps_b, list(range(N_CORES)), trace=_trace, tmpdir=tds[1]
    )

    out = np.zeros((N, CLS), np.float32)
    for c in range(N_CORES):
        out[c * NLOC : (c + 1) * NLOC] = np.asarray(
            resB.results[c]["houtT"], np.float32
        )[:CLS, :NLOC].T
    out += b2[None, :]

    kernel._last = (resA, resB)
    return out


# revision 38
# speedup vs baseline: 1.3149x; 1.0081x over previous
"""GAT (2-layer, PyG-style) on 8 Trainium2 NeuronCores.

Strategy (v2 — host-staged gather, two collective-free device programs)
----------------------------------------------------------------------
- Nodes are sharded across the 8 cores by dst (N/8 rows each). Edges are
  sorted by dst and packed into 128-dst windows; each window's edges are
  padded to 128-edge tiles (slot layout identical on all cores; per-core
  counts only differ in the padding).
- The host stages the per-edge operand tiles (the "gather"):
    program P1 gets xgT tiles  — x[src_e] columns, [128 feat x 128 edge],
    program P2 gets g2 tiles   — tab2[src_e] rows, [128 edge x CLS],
  and the per-slot alpha/dst-position planes. Attention coefficients are
  computed on the host exactly as in the v1 kernel (layer-1 alpha is a
  pure function of the inputs; layer-2 alpha comes from per-node a_s2 /
  a_d2 scalars returned by P1).
- Program P1, per 128-edge tile (inputs streamed in ~17 ramped slabs over
  4 rotating SBUF slots; per-instruction costs are HW-measured):
    g   = xgT_tile^T @ W1            (PE, psum, 8-tile psum batches)
    g   -> sbuf bf16                 (ACT copy, one op per psum batch)
    gw  = g * alpha1                 (DVE tensor_tensor over the whole
                                      chunk; the alpha plane is shipped
                                      pair-duplicated so the DVE 2x mode
                                      applies)
    oh  = (iota == dpos)             (DVE tensor_tensor, whole chunk,
                                      pair-duplicated dpos plane, 2x)
    aggT += gw^T @ oh                (PE, psum accum per 128-dst window;
                                      streaming 128 columns keeps the PE
                                      HAM at the warm 2.4 GHz p-state)
  and per window: h1T = Relu(aggT + b1) (ACT, bias per partition), then
  tab2T = [v_s2 | v_d2 | W2]^T @ h1T (PE) which is written out: rows
  0, 1 are a_s2 / a_d2, rows 2..CLS+1 are the layer-2 table.
- Program P2: g2 resident via ~9 ramped slab DMAs; per chunk one
  g2w = g2 * alpha2 and one batched one-hot (DVE, 2x mode), then
  out2T += g2w^T @ oh per tile (PE streams 128 columns -> warm p-state).
  Host adds b2 and reassembles the full output.

All engine work stays on device; the host does input marshalling
(edge-index bookkeeping, fancy-indexed tile staging) and the softmax
normalization of the attention logits, as in v1.

Self-contained: all shapes/structure are derived from the actual inputs.
"""

import numpy as np
import ml_dtypes

import bass_rust
import concourse.bass as bass
import concourse.bacc as bacc
import concourse.mybir as mybir
from concourse.bass_utils import run_bass_kernel_spmd
from concourse.tile import TileContext, ScopedClock

# ----------------------------------------------------------------------------
# Workaround: this walrus build rejects >1 sync wait on a CTRL op, but the
# stock TileContext tail drain carries one wait per live proc. Split them
# across nofuse NOPs (one wait each).
# ----------------------------------------------------------------------------


def _patched_drain_and_barrier(self, tick_clock, wait_clock):
    nc = self.nc
    probe = nc.sync.nop(nofuse=True, hint="tail_drain_waits")
    wait_clock.add_sem_waits(probe.ins, ScopedClock({None: tick_clock.global_clock}))
    si = probe.ins.sync_info
    waits = list(si.on_wait) if si is not None else []
    if len(waits) > 1:
        probe.ins.sync_info = bass_rust.SyncInfo(on_wait=waits[:1], on_update=[])
        for i in range(1, len(waits)):
            n = nc.sync.nop(nofuse=True, hint=f"tail_drain_waits_{i}")
            n.ins.sync_info = bass_rust.SyncInfo(on_wait=waits[i : i + 1], on_update=[])
    nc.sync.drain()
    nc.all_engine_barrier()
    assert self.sems is not None
    popped = nc._tile_sem_poison_stack.pop()
    assert popped is self._sem_poison
    nc.clear_and_free_semaphores(list(self.sems.allocated().values()))
    nc.all_engine_barrier()


TileContext._drain_and_barrier = _patched_drain_and_barrier

MAX_WAITS = 1  # this walrus build rejects instructions with more sync waits


def _split_sync_waits(nc, max_waits=MAX_WAITS):
    """Hoist excess per-instruction sync waits onto standalone nofuse NOPs
    placed immediately before the instruction (same engine)."""
    n_new = 0
    for bbname, bassbb in list(nc._state.bb_map.items()):
        bb = bassbb.bb
        insts = list(bb.instructions)
        out = []
        changed = False
        for inst in insts:
            si = inst.sync_info
            if si is not None and len(si.on_wait) > max_waits:
                waits = list(si.on_wait)
                extra = waits[:-max_waits]
                for j in range(0, len(extra), max_waits):
                    nop = mybir.InstNoOp(
                        name=f"{inst.name}-w{n_new}",
                        engine=inst.engine,
                        bass_nofuse=True,
                        sync_info=bass_rust.SyncInfo(
                            on_wait=extra[j : j + max_waits], on_update=[]
                        ),
                    )
                    n_new += 1
                    nc.register_instruction(nop, overwrite=True)
                    out.append(nop)
                inst.sync_info = bass_rust.SyncInfo(
                    on_wait=waits[-max_waits:], on_update=list(si.on_update)
                )
                changed = True
            out.append(inst)
        if changed:
            bb.instructions = out
    return n_new

# ----------------------------------------------------------------------------

P = 128
N_CORES = 8
NEG_SLOPE = 0.2
BATCH = 8  # tiles per psum batch in P1 (psum tile spans 2 banks)

F32 = mybir.dt.float32
BF16 = mybir.dt.bfloat16
BF = ml_dtypes.bfloat16

_CACHE = {}


def _leaky(z):
    return np.where(z > 0, z, NEG_SLOPE * z)


def _seg_softmax(z, dst, n):
    """Exact segment softmax over sorted dst (every dst has >=1 edge)."""
    starts = np.searchsorted(dst, np.arange(n))
    m = np.maximum.reduceat(z, starts, axis=0)
    w = np.exp(z - m[dst])
    den = np.add.reduceat(w, starts, axis=0)
    return w / den[dst]


class _Meta:
    pass


def _preprocess(N, edge_index):
    """Sort edges by dst, shard by dst range, build the static window/tile
    slot structure shared by both device programs (identical on all cores;
    per-core data differs only in padding)."""
    mt = _Meta()
    assert N % N_CORES == 0
    NLOC = N // N_CORES
    CH = (NLOC + P - 1) // P
    mt.N, mt.NLOC, mt.CH = N, NLOC, CH
    mt.SH_PAD = CH * P

    src = np.concatenate([edge_index[0], np.arange(N, dtype=np.int64)])
    dst = np.concatenate([edge_index[1], np.arange(N, dtype=np.int64)])
    order = np.argsort(dst, kind="stable")
    mt.src_s, mt.dst_s = src[order], dst[order]
    E = src.shape[0]
    mt.E = E

    # per-(core, window) edge ranges
    cnt = np.zeros((N_CORES, CH), dtype=np.int64)
    rng = np.zeros((N_CORES, CH, 2), dtype=np.int64)
    for c in range(N_CORES):
        for k in range(CH):
            d0 = c * NLOC + k * P
            d1 = min(c * NLOC + min((k + 1) * P, NLOC), N)
            s = np.searchsorted(mt.dst_s, d0)
            e = np.searchsorted(mt.dst_s, d1)
            rng[c, k] = (s, e)
            cnt[c, k] = e - s

    tiles_k = ((cnt.max(axis=0) + P - 1) // P).astype(np.int64)  # per window
    mt.tiles_k = tiles_k
    mt.T = int(tiles_k.sum())
    mt.S = mt.T * P
    mt.tile_off = np.concatenate([[0], np.cumsum(tiles_k)])[:-1]  # tile idx of win k

    # per-core slot arrays
    slot_src = np.zeros((N_CORES, mt.S), dtype=np.int64)
    slot_eid = np.full((N_CORES, mt.S), -1, dtype=np.int64)
    slot_dpos = np.full((N_CORES, mt.S), -1.0, dtype=np.float64)
    for c in range(N_CORES):
        for k in range(CH):
            s, e = rng[c, k]
            ne = e - s
            s0 = int(mt.tile_off[k]) * P
            slot_src[c, s0 : s0 + ne] = mt.src_s[s:e]
            slot_eid[c, s0 : s0 + ne] = np.arange(s, e)
            slot_dpos[c, s0 : s0 + ne] = mt.dst_s[s:e] - (c * NLOC + k * P)
    mt.slot_src = slot_src
    mt.slot_eid = slot_eid

    # dpos plane [C, 128, T] f32: slot j -> [j%128, j//128]
    mt.dpos_plane = np.ascontiguousarray(
        slot_dpos.reshape(N_CORES, mt.T, P).transpose(0, 2, 1)
    ).astype(np.float32)
    return mt


def _alpha_plane_pairs(mt, alpha):
    """alpha [E, H] (dst-sorted edge order) -> [C, 128, T, H, 2] bf16 slot
    planes with each value duplicated in pairs (so the DVE 2x mode applies)."""
    H = alpha.shape[1]
    eid = mt.slot_eid
    valid = eid >= 0
    vals = np.zeros((N_CORES, mt.S, H), dtype=np.float32)
    vals[valid] = alpha[eid[valid]].astype(np.float32)
    out = vals.reshape(N_CORES, mt.T, P, H).transpose(0, 2, 1, 3)  # [C,128,T,H]
    out = np.repeat(out[..., None], 2, axis=-1)  # [C,128,T,H,2]
    return np.ascontiguousarray(out).astype(BF)


def _onehot_batch(nc, oh_sb, iota_sb, dexp_sb, t0, nt):
    """oh[:, j, d] = (iota[d] == dpos[:, t0+j]) for j in [0, nt), one DVE op.
    All operands viewed with a trailing [1,2] pair dim so the 2x mode kicks
    in (dexp is the pair-duplicated dpos plane)."""
    d_ap = dexp_sb[:, t0 : t0 + nt, :]
    d_bc = bass.AP(
        d_ap.tensor,
        d_ap.offset,
        [list(d_ap.ap[0]), [2, nt], [0, P // 2], [1, 2]],
    )
    oh_ap = oh_sb[:, :nt, :]
    oh_v = bass.AP(
        oh_ap.tensor,
        oh_ap.offset,
        [list(oh_ap.ap[0]), [P, nt], [2, P // 2], [1, 2]],
    )
    io_ap = iota_sb[:]
    io_bc = bass.AP(
        io_ap.tensor,
        io_ap.offset,
        [list(io_ap.ap[0]), [0, nt], [2, P // 2], [1, 2]],
    )
    nc.vector.tensor_tensor(
        out=oh_v, in0=io_bc, in1=d_bc, op=mybir.AluOpType.is_equal
    )


def _build_p1(mt, F_IN, HEADS, HID, CLS):
    """Layer-1 program: xgT tiles -> h1 windows -> tab2T/asd output."""
    F_HID = HEADS * HID
    CH, T, S = mt.CH, mt.T, mt.S
    W2E = ((CLS + 2 + 15) // 16) * 16  # padded [v_s2 | v_d2 | W2] columns
    assert F_IN == P and F_HID == P

    nc = bacc.Bacc("TRN2", target_bir_lowering=False, debug=False, num_devices=N_CORES)
    xg_in = nc.declare_dram_parameter("xg", [P, S], BF16, isOutput=False)
    aexp_in = nc.declare_dram_parameter("aexp", [P, T * HEADS * 2], BF16, isOutput=False)
    dexp_in = nc.declare_dram_parameter("dexp", [P, T * 2], BF16, isOutput=False)
    iota_in = nc.declare_dram_parameter("iota", [P, P], BF16, isOutput=False)
    w1_in = nc.declare_dram_parameter("w1", [P, F_HID], BF16, isOutput=False)
    w2e_in = nc.declare_dram_parameter("w2e", [P, W2E], BF16, isOutput=False)
    b1_in = nc.declare_dram_parameter("b1col", [P, 1], F32, isOutput=False)
    houtT = nc.declare_dram_parameter("houtT", [W2E, CH * P], BF16, isOutput=True)
    asdT = nc.declare_dram_parameter("asdT", [2, CH * P], F32, isOutput=True)

    # streamed-xg slab boundaries: ~17 DMAs, 4 rotating SBUF slots; the first
    # slabs are small so the first chunk's data lands quickly.
    slab_bounds = [0]
    tgt_full = (T + 15) // 16
    for k in range(CH):
        t_end = int(mt.tile_off[k]) + int(mt.tiles_k[k])
        n_so_far = len(slab_bounds) - 1
        tgt = tgt_full // 4 if n_so_far < 2 else (tgt_full // 2 if n_so_far < 3 else tgt_full)
        if (t_end - slab_bounds[-1] >= tgt or k == CH - 1) and t_end > slab_bounds[-1]:
            slab_bounds.append(t_end)
    chunk_slab = {}
    for k in range(CH):
        t0 = int(mt.tile_off[k])
        chunk_slab[k] = sum(1 for b in slab_bounds[1:-1] if b <= t0)

    with TileContext(nc) as tc:
        with (
            tc.tile_pool(name="res", bufs=1) as res,
            tc.tile_pool(name="g4p", bufs=4) as g4p,
            tc.tile_pool(name="gwp", bufs=4) as gwp,
            tc.tile_pool(name="ohp", bufs=4) as ohp,
            tc.tile_pool(name="epi", bufs=3) as epi,
            tc.tile_pool(name="psg", bufs=2, space="PSUM") as psg,
            tc.tile_pool(name="psa", bufs=2, space="PSUM") as psa,
            tc.tile_pool(name="pst", bufs=2, space="PSUM") as pst,
        ):
            # residents on the ACT HWDGE ring; data slabs on the SP ring, so
            # the first slab isn't queued behind the big alpha plane.
            w1_sb = res.tile([P, F_HID], BF16)
            nc.scalar.dma_start(out=w1_sb[:], in_=w1_in[:])
            iota_sb = res.tile([P, P], BF16)
            nc.scalar.dma_start(out=iota_sb[:], in_=iota_in[:])
            b1_sb = res.tile([P, 1], F32)
            nc.scalar.dma_start(out=b1_sb[:], in_=b1_in[:])
            dexp_sb = res.tile([P, T, 2], BF16)
            nc.scalar.dma_start(
                out=dexp_sb[:],
                in_=dexp_in[:].rearrange("p (t two) -> p t two", two=2),
            )
            aexp_sb = res.tile([P, T, HEADS, 2], BF16)
            nc.scalar.dma_start(
                out=aexp_sb[:],
                in_=aexp_in[:].rearrange("p (t h two) -> p t h two", h=HEADS, two=2),
            )
            w2e_sb = res.tile([P, W2E], BF16)
            nc.scalar.dma_start(out=w2e_sb[:], in_=w2e_in[:])

            max_slab = max(
                slab_bounds[i + 1] - slab_bounds[i]
                for i in range(len(slab_bounds) - 1)
            )
            xg_slabs = []
            for si in range(len(slab_bounds) - 1):
                a, b = slab_bounds[si], slab_bounds[si + 1]
                sl = res.tile(
                    [P, max_slab, P], BF16, name=f"xgsl{si}", tag=f"xgsl{si % 4}"
                )[:, : b - a, :]
                nc.sync.dma_start(
                    out=sl[:],
                    in_=xg_in[:, a * P : b * P].rearrange("p (t e) -> p t e", e=P),
                )
                xg_slabs.append(sl)

            for k in range(CH):
                nt = int(mt.tiles_k[k])
                t0 = int(mt.tile_off[k])
                si = chunk_slab[k]
                xg_sb = xg_slabs[si][
                    :, t0 - slab_bounds[si] : t0 - slab_bounds[si] + nt, :
                ]
                agg_ps = psa.tile([P, P], F32, tag="agg")
                g_sb = g4p.tile([P, nt, P], BF16, tag="g")
                n_batches = (nt + BATCH - 1) // BATCH
                for b in range(n_batches):
                    j0 = b * BATCH
                    nb = min(BATCH, nt - j0)
                    g_ps = psg.tile([P, BATCH * P], F32, tag="gps")
                    for j in range(nb):
                        nc.tensor.matmul(
                            g_ps[:, j * P : (j + 1) * P],
                            lhsT=xg_sb[:, j0 + j, :],
                            rhs=w1_sb[:],
                            start=True,
                            stop=True,
                            skip_group_check=True,
                        )
                    nc.scalar.activation(
                        g_sb[:, j0 : j0 + nb, :],
                        g_ps[:, : nb * P].rearrange("p (t e) -> p t e", e=P),
                        mybir.ActivationFunctionType.Copy,
                    )
                # gw = g * alpha, split at the psum-batch boundary so the
                # aggregation matmuls start before the last copy lands
                gw = gwp.tile([P, nt, P], BF16, tag="gw")
                def gview(tile, lo, n):
                    sl = tile[:, lo : lo + n, :]
                    return bass.AP(
                        sl.tensor,
                        sl.offset,
                        [list(sl.ap[0]), [HID, n * HEADS], [2, HID // 2], [1, 2]],
                    )
                mult_splits = [(0, min(BATCH, nt))]
                if nt > BATCH:
                    mult_splits.append((BATCH, nt - BATCH))
                for lo, n in mult_splits:
                    a_ap = aexp_sb[:, t0 + lo : t0 + lo + n, :, :]
                    a_bc = bass.AP(
                        a_ap.tensor,
                        a_ap.offset,
                        [list(a_ap.ap[0]), [2, n * HEADS], [0, HID // 2], [1, 2]],
                    )
                    nc.vector.tensor_tensor(
                        out=gview(gw, lo, n),
                        in0=gview(g_sb, lo, n),
                        in1=a_bc,
                        op=mybir.AluOpType.mult,
                    )
                oh = ohp.tile([P, nt, P], BF16, tag="oh")
                _onehot_batch(nc, oh, iota_sb, dexp_sb, t0, nt)
                for j in range(nt):
                    nc.tensor.matmul(
                        agg_ps[:],
                        lhsT=gw[:, j, :],
                        rhs=oh[:, j, :],
                        start=(j == 0),
                        stop=(j == nt - 1),
                        skip_group_check=True,
                    )
                # epilogue: h1T = relu(aggT + b1), tab2T = W2e^T @ h1T
                h1T = epi.tile([P, P], BF16, tag="h1T")
                nc.scalar.activation(
                    h1T[:],
                    agg_ps[:],
                    mybir.ActivationFunctionType.Relu,
                    bias=b1_sb[:, 0:1],
                )
                t2_ps = pst.tile([W2E, P], F32, tag="t2")
                nc.tensor.matmul(
                    t2_ps[:],
                    lhsT=w2e_sb[:],
                    rhs=h1T[:],
                    start=True,
                    stop=True,
                    skip_group_check=True,
                )
                t2_sb = epi.tile([W2E, P], BF16, tag="t2sb")
                nc.vector.tensor_copy(out=t2_sb[:], in_=t2_ps[:])
                asd_sb = epi.tile([2, P], F32, tag="asd")
                nc.vector.tensor_copy(out=asd_sb[:], in_=t2_ps[0:2, :])
                nc.sync.dma_start(out=houtT[:, k * P : (k + 1) * P], in_=t2_sb[:])
                nc.sync.dma_start(out=asdT[:, k * P : (k + 1) * P], in_=asd_sb[:])
    nc.compile()
    _split_sync_waits(nc)
    return nc


def _build_p2(mt, CLS):
    """Layer-2 program: g2 tiles * alpha2 aggregated per window (transposed
    matmul so the PE streams 128 columns and leaves the cold p-state)."""
    CH, T, S = mt.CH, mt.T, mt.S
    CLSP = ((CLS + 15) // 16) * 16

    nc = bacc.Bacc("TRN2", target_bir_lowering=False, debug=False, num_devices=N_CORES)
    g2_in = nc.declare_dram_parameter("g2", [P, T * CLSP], BF16, isOutput=False)
    aexp_in = nc.declare_dram_parameter("aexp", [P, T * 2], BF16, isOutput=False)
    dexp_in = nc.declare_dram_parameter("dexp", [P, T * 2], BF16, isOutput=False)
    iota_in = nc.declare_dram_parameter("iota", [P, P], BF16, isOutput=False)
    houtT = nc.declare_dram_parameter("houtT", [CLSP, CH * P], F32, isOutput=True)

    # resident-g2 slab boundaries: ~4 big DMAs aligned to chunk starts
    slab_bounds = [0]
    tgt_full = (T + 7) // 8
    for k in range(CH):
        t_end = int(mt.tile_off[k]) + int(mt.tiles_k[k])
        n_so_far = len(slab_bounds) - 1
        tgt = tgt_full // 4 if n_so_far < 2 else (tgt_full // 2 if n_so_far < 3 else tgt_full)
        if (t_end - slab_bounds[-1] >= tgt or k == CH - 1) and t_end > slab_bounds[-1]:
            slab_bounds.append(t_end)
    chunk_slab = {}
    for k in range(CH):
        t0 = int(mt.tile_off[k])
        chunk_slab[k] = sum(1 for b in slab_bounds[1:-1] if b <= t0)

    with TileContext(nc) as tc:
        with (
            tc.tile_pool(name="res", bufs=1) as res,
            tc.tile_pool(name="gwp", bufs=4) as gwp,
            tc.tile_pool(name="ohp", bufs=4) as ohp,
            tc.tile_pool(name="epi", bufs=3) as epi,
            tc.tile_pool(name="pso", bufs=3, space="PSUM") as pso,
        ):
            iota_sb = res.tile([P, P], BF16)
            nc.scalar.dma_start(out=iota_sb[:], in_=iota_in[:])
            dexp_sb = res.tile([P, T, 2], BF16)
            nc.scalar.dma_start(
                out=dexp_sb[:],
                in_=dexp_in[:].rearrange("p (t two) -> p t two", two=2),
            )
            aexp_sb = res.tile([P, T, 2], BF16)
            nc.scalar.dma_start(
                out=aexp_sb[:],
                in_=aexp_in[:].rearrange("p (t two) -> p t two", two=2),
            )
            g2_slabs = []
            for si in range(len(slab_bounds) - 1):
                a, b = slab_bounds[si], slab_bounds[si + 1]
                sl = res.tile([P, b - a, CLSP], BF16, name=f"g2sl{si}", tag=f"g2sl{si}")
                nc.sync.dma_start(
                    out=sl[:],
                    in_=g2_in[:, a * CLSP : b * CLSP].rearrange(
                        "p (t e) -> p t e", e=CLSP
                    ),
                )
                g2_slabs.append(sl)

            for k in range(CH):
                nt = int(mt.tiles_k[k])
                t0 = int(mt.tile_off[k])
                si = chunk_slab[k]
                g2_sb = g2_slabs[si][:, t0 - slab_bounds[si] : t0 - slab_bounds[si] + nt, :]
                # g2w = g2 * alpha2 (one DVE op per chunk, 2x mode)
                g2w = gwp.tile([P, nt, CLSP], BF16, tag="g2w")
                a_ap = aexp_sb[:, t0 : t0 + nt, :]
                a_bc = bass.AP(
                    a_ap.tensor,
                    a_ap.offset,
                    [list(a_ap.ap[0]), [2, nt], [0, CLSP // 2], [1, 2]],
                )
                gview = lambda tile: bass.AP(
                    tile.tensor,
                    tile.offset,
                    [list(tile.ap[0]), [CLSP, nt], [2, CLSP // 2], [1, 2]],
                )
                nc.vector.tensor_tensor(
                    out=gview(g2w[:, :, :]),
                    in0=gview(g2_sb[:, :, :]),
                    in1=a_bc,
                    op=mybir.AluOpType.mult,
                )
                oh = ohp.tile([P, nt, P], BF16, tag="oh")
                _onehot_batch(nc, oh, iota_sb, dexp_sb, t0, nt)
                o_ps = pso.tile([CLSP, P], F32, tag="o")
                for j in range(nt):
                    nc.tensor.matmul(
                        o_ps[:],
                        lhsT=g2w[:, j, :],
                        rhs=oh[:, j, :],
                        start=(j == 0),
                        stop=(j == nt - 1),
                        skip_group_check=True,
                    )
                o_sb = epi.tile([CLSP, P], F32, tag="osb")
                nc.scalar.activation(
                    o_sb[:], o_ps[:], mybir.ActivationFunctionType.Copy
                )
                nc.sync.dma_start(out=houtT[:, k * P : (k + 1) * P], in_=o_sb[:])
    nc.compile()
    _split_sync_waits(nc)
    return nc


def kernel(
    x,
    edge_index,
    W1,
    att_src1,
    att_dst1,
    b1,
    W2,
    att_src2,
    att_dst2,
    b2,
    _trace=False,
    _tmpdirs=None,
):
    x = np.asarray(x, dtype=np.float32)
    edge_index = np.asarray(edge_index).astype(np.int64)
    W1 = np.asarray(W1, dtype=np.float32)
    att_src1 = np.asarray(att_src1, dtype=np.float32)
    att_dst1 = np.asarray(att_dst1, dtype=np.float32)
    b1 = np.asarray(b1, dtype=np.float32)
    W2 = np.asarray(W2, dtype=np.float32)
    att_src2 = np.asarray(att_src2, dtype=np.float32)
    att_dst2 = np.asarray(att_dst2, dtype=np.float32)
    b2 = np.asarray(b2, dtype=np.float32)

    N, F_IN = x.shape
    HEADS, HID = att_src1.shape
    CLS = W2.shape[1]
    W2E = ((CLS + 2 + 15) // 16) * 16
    CLSP = ((CLS + 15) // 16) * 16

    key = (N, edge_index.shape[1], F_IN, HEADS, HID, CLS, hash(edge_index.tobytes()))
    if key in _CACHE:
        mt, ncA, ncB = _CACHE[key]
    else:
        mt = _preprocess(N, edge_index)
        ncA = _build_p1(mt, F_IN, HEADS, HID, CLS)
        ncB = _build_p2(mt, CLS)
        _CACHE[key] = (mt, ncA, ncB)

    NLOC, CH, T, S = mt.NLOC, mt.CH, mt.T, mt.S

    # ---- host: layer-1 alpha (a_s/a_d are linear in x) ----
    W1r = W1.reshape(F_IN, HEADS, HID)
    v_s = np.einsum("fhc,hc->fh", W1r, att_src1)
    v_d = np.einsum("fhc,hc->fh", W1r, att_dst1)
    a_s = x.astype(np.float64) @ v_s.astype(np.float64)
    a_d = x.astype(np.float64) @ v_d.astype(np.float64)
    z1 = _leaky(a_s[mt.src_s] + a_d[mt.dst_s])
    alpha1 = _seg_softmax(z1, mt.dst_s, N)

    aexp = _alpha_plane_pairs(mt, alpha1)  # [C,128,T,H,2] bf16
    iota = np.tile(np.arange(P, dtype=np.float32)[None, :], (P, 1)).astype(BF)
    dexp = np.ascontiguousarray(
        np.repeat(mt.dpos_plane[..., None], 2, axis=-1)
    ).astype(BF)  # [C,128,T,2]

    # xgT tiles: [C, 128 feat, S] bf16 = x.T columns at slot srcs
    xT16 = np.ascontiguousarray(x.astype(BF).T)  # [F_IN, N]
    w1b = W1.astype(BF)
    v_s2 = (W2 @ att_src2[0]).astype(np.float32)
    v_d2 = (W2 @ att_dst2[0]).astype(np.float32)
    w2e = np.zeros((HEADS * HID, W2E), np.float32)
    w2e[:, 0] = v_s2
    w2e[:, 1] = v_d2
    w2e[:, 2 : 2 + CLS] = W2
    w2eb = w2e.astype(BF)
    b1col = b1.reshape(P, 1).astype(np.float32)

    in_maps_a = []
    for c in range(N_CORES):
        xg = np.ascontiguousarray(xT16[:, mt.slot_src[c]])  # [128, S]
        in_maps_a.append(
            {
                "xg": xg,
                "aexp": np.ascontiguousarray(aexp[c].reshape(P, -1)),
                "dexp": np.ascontiguousarray(dexp[c].reshape(P, -1)),
                "iota": iota,
                "w1": w1b,
                "w2e": w2eb,
                "b1col": b1col,
            }
        )

    tds = _tmpdirs or [None, None]
    resA = run_bass_kernel_spmd(
        ncA, in_maps_a, list(range(N_CORES)), trace=_trace, tmpdir=tds[0]
    )

    # host: assemble tab2 + a_s2/a_d2, compute alpha2
    tab2 = np.zeros((N, CLSP), BF)
    asd = np.zeros((N, 2), np.float64)
    for c in range(N_CORES):
        hT = np.asarray(resA.results[c]["houtT"])  # [W2E, CH*P] bf16
        tab2[c * NLOC : (c + 1) * NLOC, :CLS] = hT[2 : 2 + CLS, :NLOC].T
        asd[c * NLOC : (c + 1) * NLOC] = np.asarray(
            resA.results[c]["asdT"], np.float64
        )[:, :NLOC].T

    z2 = _leaky(asd[mt.src_s, 0] + asd[mt.dst_s, 1])[:, None]
    alpha2 = _seg_softmax(z2, mt.dst_s, N)
    a2exp = _alpha_plane_pairs(mt, alpha2)  # [C,128,T,1,2]

    in_maps_b = []
    for c in range(N_CORES):
        g2 = tab2[mt.slot_src[c]]  # [S, CLSP] bf16
        g2 = np.ascontiguousarray(
            g2.reshape(T, P, CLSP).transpose(1, 0, 2).reshape(P, T * CLSP)
        )
        in_maps_b.append(
            {
                "g2": g2,
                "aexp": np.ascontiguousarray(a2exp[c].reshape(P, -1)),
                "dexp": np.ascontiguousarray(dexp[c].reshape(P, -1)),
                "iota": iota,
            }
        )

    resB = run_bass_kernel_spmd(
        ncB, in_maps_b, list(range(N_CORES)), trace=_trace, tmpdir=tds[1]
    )

    out = np.zeros((N, CLS), np.float32)
    for c in range(N_CORES):
        out[c * NLOC : (c + 1) * NLOC] = np.asarray(
            resB.results[c]["houtT"], np.float32
        )[:CLS, :NLOC].T
    out += b2[None, :]

    kernel._last = (resA, resB)
    return out
